# revision 1
# baseline (speedup 1.0000x reference)
"""GPT-2 attention block on 8 TRN2 NeuronCores.

Sharding (Megatron-style): core i owns heads (2i, 2i+1) for both batches.
 - QKV projection computed transposed: qkvT = Wshard^T @ X^T  -> [384, 4096]
   (rows: q0|q1|k0|k1|v0|v1 head-dim slices, cols: tokens b-major).
   X^T is DMA'd in token chunks so each chunk's projection (full-D
   contraction) completes as the chunk lands.
 - scores per (batch, head) in transposed layout S^T[k, q], causal tiles
   only, both heads side by side in one 2-bank PSUM tile; one fused exp on
   ScalarE per tile (the 1/sqrt(64) scale folded into the activation).
 - AV via matmul with ones-augmented V ([v_h | 1] from PE transposes of
   V^T) as the stationary operand: row 64 of the psum accumulates the
   softmax denominator for free.
 - av^T is transposed back to token-major via PE, normalized per token
   (reciprocal denominators reach per-partition layout via a DRAM bounce),
   then AllToAlls reshard to sequence parallelism and each core runs the
   full output projection for its tokens. Batch 0 reshards in one
   AllToAll (256-token blocks; overlaps batch 1's attention); batch 1 in
   two half-batch AllToAlls (128-token blocks) so the first half also
   overlaps attention and only the second half sits in the tail.
Output per core j: [512, 1024] fp32 — rows 0:256 = batch0 tokens 256j..,
rows 256:384 = batch1 tokens 128j.., rows 384:512 = batch1 tokens
1024+128j..; host reassembles. Matmuls in bf16 (fp32 PSUM accumulation);
softmax in fp32. Post passes: ldweights dedup + splitting multi-wait
instructions into single-wait NoOps (this walrus build caps HW waits at 1).
"""

import numpy as np
import ml_dtypes

import concourse.bass as bass
import concourse.mybir as mybir
import concourse.tile as tile
from concourse.bass_utils import run_bass_kernel_spmd

BF16 = mybir.dt.bfloat16
F32 = mybir.dt.float32
AF = mybir.ActivationFunctionType

B, S, D, H = 2, 2048, 1024, 16
NT = B * S          # 4096 tokens, b-major
NCORES = 8
DK = D // H         # 64
NEG = -1.0e30
SCALE = 0.125       # 1/sqrt(64)

_CACHE = {}
_NO_COLLECTIVE = False


def _build(debug_dumps=False):
    nc = bass.Bass("TRN2", target_bir_lowering=False, debug=False,
                   num_devices=NCORES)

    xT = nc.dram_tensor("xT", [D, NT], BF16, kind="ExternalInput").ap()
    wqkv = nc.dram_tensor("wqkv", [D, 384], BF16, kind="ExternalInput").ap()
    wp = nc.dram_tensor("wp", [D, D], BF16, kind="ExternalInput").ap()
    cbf16 = nc.dram_tensor("cbf16", [128, 129], BF16, kind="ExternalInput").ap()
    cf32 = nc.dram_tensor("cf32", [128, 131], F32, kind="ExternalInput").ap()
    out = nc.dram_tensor("out", [512, 1024], F32, kind="ExternalOutput").ap()
    dbg = None
    if debug_dumps:
        dbg = {
            "dbg_qkvT": nc.dram_tensor(
                "dbg_qkvT", [128, 3, NT], BF16, kind="ExternalOutput").ap(),
            "dbg_av": nc.dram_tensor(
                "dbg_av", [128, 32, 128], BF16, kind="ExternalOutput").ap(),
            "dbg_den": nc.dram_tensor(
                "dbg_den", [8, 128, 8], F32, kind="ExternalOutput").ap(),
            "dbg_aT": nc.dram_tensor(
                "dbg_aT", [128, 8, 512], BF16, kind="ExternalOutput").ap(),
        }

    with tile.TileContext(nc) as tc:
        _body(tc, out, xT, wqkv, wp, cbf16, cf32, dbg)
    _dedup_ldweights(nc)
    _split_multi_waits(nc)
    return nc


def _dedup_ldweights(nc):
    """Drop a back-to-back identical, wait-free Ldweights (weights already
    resident; only Matmults in between; transposes clobber -> reset)."""
    for f in nc.m.functions:
        for bb in f.blocks:
            insts = bb.instructions
            new = []
            changed = False
            last_w = None
            for inst in insts:
                nm = inst.__class__.__name__
                if getattr(inst, "engine", None) == mybir.EngineType.PE:
                    if nm == "InstLdweights":
                        si = inst.sync_info
                        key = repr(inst.ins)
                        no_waits = si is None or not si.on_wait
                        no_upd = si is None or not si.on_update
                        if key == last_w and no_waits and no_upd:
                            changed = True
                            continue  # drop duplicate load
                        last_w = key
                    elif nm == "InstMatmult":
                        if getattr(inst, "is_transpose", False):
                            last_w = None
                    else:
                        last_w = None
                new.append(inst)
            if changed:
                bb.instructions = new


def _split_multi_waits(nc):
    """Walrus caps HW sync waits at 1 per instruction: hoist extras onto
    dedicated NoOps inserted just before the offender (same engine queue)."""
    import bass_rust
    nid = [0]
    for f in nc.m.functions:
        for bb in f.blocks:
            insts = bb.instructions
            new = []
            changed = False
            for inst in insts:
                si = getattr(inst, "sync_info", None)
                if si is not None and len(si.on_wait) > 1:
                    changed = True
                    waits = list(si.on_wait)
                    for w in waits[:-1]:
                        nid[0] += 1
                        nop = mybir.InstNoOp(
                            name=f"I-waitnop-{nid[0]}", ins=[], outs=[])
                        nop.engine = inst.engine
                        nop.sync_info = bass_rust.SyncInfo(
                            on_wait=[w], on_update=[])
                        new.append(nop)
                    inst.sync_info = bass_rust.SyncInfo(
                        on_wait=[waits[-1]], on_update=list(si.on_update))
                new.append(inst)
            if changed:
                bb.instructions = new


def _body(tc, out, xT, wqkv, wp, cbf16, cf32, dbg=None):
    nc = tc.nc

    with (
        tc.tile_pool(name="persist", bufs=1) as persist,
        tc.tile_pool(name="expp", bufs=24) as expp_pool,
        tc.tile_pool(name="avts", bufs=2) as avts_pool,
        tc.tile_pool(name="dens", bufs=2) as dens_pool,
        tc.tile_pool(name="smalls", bufs=4) as smalls_pool,
        tc.tile_pool(name="ablk", bufs=2) as ablk_pool,
        tc.tile_pool(name="outs", bufs=3) as outs_pool,
        tc.tile_pool(name="pss", bufs=2, space="PSUM") as pss_pool,
        tc.tile_pool(name="qkvp", bufs=2, space="PSUM") as qkvp_pool,
        tc.tile_pool(name="psa", bufs=2, space="PSUM") as psa_pool,
        tc.tile_pool(name="dram", bufs=1, space="DRAM") as dram_pool,
    ):
        # ---- persistent SBUF ----
        xT_sb = persist.tile([128, 8, NT], BF16)        # X^T, D-tile major
        wqkv_sb = persist.tile([128, 8, 384], BF16)
        wp_sb = persist.tile([128, 8, 1024], BF16)
        qkvT_sb = persist.tile([128, 3, NT], BF16)      # q|k|v ^T rows
        v_aug = persist.tile([128, 32, 130], BF16)      # [v_h0|1|v_h1|1] per token-tile
        av_sb = persist.tile([128, 32, 128], BF16)      # av natural, per token-tile
        aT_sb = persist.tile([128, 8, 512], BF16)       # a^T after all-to-all
        cbf16_sb = persist.tile([128, 129], BF16)
        cf32_sb = persist.tile([128, 131], F32)
        ident_sb = cbf16_sb[:, 0:128]
        ones_sb = cbf16_sb[:, 128:129]
        maskT_sb = cf32_sb[:, 0:128]
        bqkv_sb = cf32_sb[:, 128:131]

        av_bounce = {0: dram_pool.tile([S, 128], BF16, name="avb0"),
                     "1a": dram_pool.tile([S // 2, 128], BF16, name="avb1a"),
                     "1b": dram_pool.tile([S // 2, 128], BF16, name="avb1b")}
        recv_bounce = {0: dram_pool.tile([S, 128], BF16, name="rcv0"),
                       "1a": dram_pool.tile([S // 2, 128], BF16, name="rcv1a"),
                       "1b": dram_pool.tile([S // 2, 128], BF16, name="rcv1b")}

        # ones columns of v_aug (rest overwritten by V transposes)
        nc.vector.memset(v_aug[:, :, 64:65], 1.0)
        nc.vector.memset(v_aug[:, :, 129:130], 1.0)

        # ---- ACT warmup: attach table-load pseudos to wait-free instructions
        warm = smalls_pool.tile([1, 2], F32, tag="warm")
        nc.vector.memset(warm[:, 0:1], 0.0)
        nc.scalar.activation(warm[:, 1:2], warm[:, 0:1], AF.Identity)
        nc.scalar.activation(warm[:, 1:2], warm[:, 0:1], AF.Exp)
        nc.scalar.activation(warm[:, 1:2], warm[:, 0:1], AF.Copy)

        # ---- input DMAs: xT chunked by TOKENS so each chunk's qkv (full
        # D contraction) completes as soon as that chunk lands.
        nc.sync.dma_start(wqkv_sb[:, :, :],
                          wqkv.rearrange("(kt p) n -> p kt n", p=128))
        nc.sync.dma_start(cbf16_sb[:, :], cbf16[:, :])
        nc.sync.dma_start(cf32_sb[:, :], cf32[:, :])
        for n in range(8):
            nc.sync.dma_start(
                xT_sb[:, :, n * 512:(n + 1) * 512],
                xT[:, n * 512:(n + 1) * 512]
                .rearrange("(kt p) w -> p kt w", p=128))
        nc.sync.dma_start(wp_sb[:, :, :],
                          wp.rearrange("(kt p) n -> p kt n", p=128))

        # ---- phase 1: qkvT = Wshard^T @ X^T, bias add, bf16 ----
        # token-chunk outer; V transposes interleaved per chunk.
        for n in range(8):
            for m in range(3):
                ps = qkvp_pool.tile([128, 512], F32, tag="qkvp")
                for kt in range(8):
                    nc.tensor.matmul(
                        ps[:, :],
                        wqkv_sb[:, kt, m * 128:(m + 1) * 128],
                        xT_sb[:, kt, n * 512:(n + 1) * 512],
                        start=(kt == 0), stop=(kt == 7),
                    )
                nc.vector.tensor_scalar_add(
                    qkvT_sb[:, m, n * 512:(n + 1) * 512],
                    ps[:, :], bqkv_sb[:, m:m + 1])
            for t in range(n * 4, n * 4 + 4):
                ps_t = psa_pool.tile([128, 128], BF16, tag="psa")
                nc.tensor.transpose(
                    ps_t[:, :], qkvT_sb[:, 2, t * 128:(t + 1) * 128],
                    ident_sb[:, :])
                nc.vector.tensor_copy(v_aug[:, t, 0:64], ps_t[:, 0:64])
                nc.vector.tensor_copy(v_aug[:, t, 65:129], ps_t[:, 64:128])

        def attention(b, hooks=None, order=(0, 1, 2, 3)):
            tok0 = b * S
            for c in order:
                nk = 4 * c + 4        # k-tiles 0..nk-1
                q0 = tok0 + c * 512   # global col of chunk start
                expp = {}
                for ki in range(nk):
                    off = max(0, (ki - 4 * c)) * 128
                    w = 512 - off
                    ps_s = pss_pool.tile([128, 2, 512], F32, tag="pss")
                    for h in range(2):
                        hp = h * 64
                        nc.tensor.matmul(
                            ps_s[:, h, :w],
                            qkvT_sb[hp:hp + 64, 1,
                                    tok0 + ki * 128: tok0 + (ki + 1) * 128],
                            qkvT_sb[hp:hp + 64, 0, q0 + off: q0 + 512],
                            start=True, stop=True,
                        )
                    if ki >= 4 * c:  # diagonal tile: causal mask on first 128
                        for h in range(2):
                            nc.vector.tensor_add(
                                ps_s[:, h, 0:128], ps_s[:, h, 0:128],
                                maskT_sb[:, :])
                    et = expp_pool.tile([128, 2, 512], BF16, tag="expp")
                    nc.scalar.activation(
                        et[:, :, :w], ps_s[:, :, :w], AF.Exp, scale=SCALE)
                    expp[ki] = (et, off, w)

                # denominator + AV (col-packed heads for AV)
                den_sb = dens_pool.tile([1, 1024], F32, tag="dens")
                avT_sbuf = avts_pool.tile([128, 512], BF16, tag="avts")
                for h in range(2):
                    avh_ps = psa_pool.tile([65, 512], F32, tag="psa")
                    for ki in range(nk):
                        et, off, w = expp[ki]
                        nc.tensor.matmul(
                            avh_ps[:, off:512],
                            v_aug[:, b * 16 + ki, h * 65:(h + 1) * 65],
                            et[:, h, :w],
                            start=(ki == 0), stop=(ki == nk - 1),
                        )
                    nc.vector.tensor_copy(
                        avT_sbuf[h * 64:(h + 1) * 64, :], avh_ps[0:64, :])
                    nc.vector.tensor_copy(den_sb[0:1, h * 512:(h + 1) * 512],
                                          avh_ps[64:65, :])

                # reshape denominators [1, h*512+qt*128+p] -> [p, h*4+qt]
                # (via DRAM: SBUF APs cannot move free offsets onto partitions)
                den_dram = dram_pool.tile([1, 1024], F32, tag="dend", bufs=2)
                nc.sync.dma_start(den_dram[:, :], den_sb[0:1, :])
                den_col = smalls_pool.tile([128, 8], F32, tag="denc")
                nc.sync.dma_start(
                    den_col[:, :],
                    den_dram.rearrange("a (hq p) -> (a p) hq", p=128))
                recip_col = smalls_pool.tile([128, 8], F32, tag="recipc")
                nc.vector.reciprocal(recip_col[:, :], den_col[:, :])
                if dbg is not None:
                    nc.sync.dma_start(dbg["dbg_den"][b * 4 + c, :, :],
                                      den_col[:, :])

                # transpose av^T -> natural, normalize per head
                for qt in range(4):
                    ps_t = psa_pool.tile([128, 128], BF16, tag="psa")
                    nc.tensor.transpose(
                        ps_t[:, :], avT_sbuf[:, qt * 128:(qt + 1) * 128],
                        ident_sb[:, :])
                    tindex = b * 16 + c * 4 + qt
                    for h in range(2):
                        hp = h * 64
                        nc.vector.tensor_scalar_mul(
                            av_sb[:, tindex, hp:hp + 64], ps_t[:, hp:hp + 64],
                            recip_col[:, h * 4 + qt: h * 4 + qt + 1])
                if hooks and c in hooks:
                    hooks[c]()

        def a2a(key, t0, nt):
            # all-to-all over av token-tiles [t0, t0+nt): nt*16-token blocks
            nc.sync.dma_start(
                av_bounce[key].rearrange("(t p) d -> p t d", p=128),
                av_sb[:, t0:t0 + nt, :])
            if _NO_COLLECTIVE:
                nc.sync.dma_start(recv_bounce[key][:, :], av_bounce[key][:, :])
            else:
                nc.gpsimd.collective_compute(
                    "AllToAll", mybir.AluOpType.bypass,
                    replica_groups=[list(range(NCORES))],
                    ins=[av_bounce[key][:, :].opt()],
                    outs=[recv_bounce[key][:, :].opt()],
                )

        def recv_stage(key, jn, col0):
            # rebuild a^T [dcol, jn*128 tokens] at aT_sb cols [col0, ...)
            a_stg = ablk_pool.tile([128, 8 * jn, 128], BF16, tag="ablk",
                                   name=f"astg{key}")
            nc.sync.dma_start(
                a_stg[:, :, :],
                recv_bounce[key].rearrange("(t p) d -> p t d", p=128))
            for s in range(8):
                for j in range(jn):
                    ps_t = qkvp_pool.tile([128, 128], BF16, tag="qkvp")
                    nc.tensor.transpose(ps_t[:, :], a_stg[:, s * jn + j, :],
                                        ident_sb[:, :])
                    nc.vector.tensor_copy(
                        aT_sb[:, s, col0 + j * 128: col0 + (j + 1) * 128],
                        ps_t[:, :])

        def proj(b, mts=(0, 1)):
            for mt in mts:
                r0 = b * 256 + mt * 128
                for n2 in range(2):
                    ps = qkvp_pool.tile([128, 512], F32, tag="qkvp")
                    for s in range(8):
                        nc.tensor.matmul(
                            ps[:, :],
                            aT_sb[:, s, r0:r0 + 128],
                            wp_sb[:, s, n2 * 512:(n2 + 1) * 512],
                            start=(s == 0), stop=(s == 7),
                        )
                    o_sb = outs_pool.tile([128, 512], F32, tag="outs")
                    nc.vector.tensor_copy(o_sb[:, :], ps[:, :])
                    nc.sync.dma_start(
                        out[r0:r0 + 128, n2 * 512:(n2 + 1) * 512],
                        o_sb[:, :])

        attention(0, hooks={3: lambda: a2a(0, 0, 16)})
        attention(1, hooks={1: lambda: a2a("1a", 16, 8),
                            3: lambda: a2a("1b", 24, 8)})
        recv_stage(0, 2, 0)
        proj(0)
        recv_stage("1a", 1, 256)
        proj(1, mts=(0,))
        recv_stage("1b", 1, 384)
        proj(1, mts=(1,))

        if dbg is not None:
            nc.sync.dma_start(
                dbg["dbg_qkvT"].rearrange("p m n -> p (m n)"),
                qkvT_sb[:, :, :].rearrange("p m n -> p (m n)"))
            nc.sync.dma_start(
                dbg["dbg_av"].rearrange("p t d -> p (t d)"),
                av_sb[:, :, :].rearrange("p t d -> p (t d)"))
            nc.sync.dma_start(
                dbg["dbg_aT"].rearrange("p s n -> p (s n)"),
                aT_sb[:, :, :].rearrange("p s n -> p (s n)"))


def _prep_inputs(hidden_states, c_attn_w, c_attn_b, c_proj_w):
    bf16 = ml_dtypes.bfloat16
    x = np.asarray(hidden_states, dtype=np.float32).reshape(NT, D)
    xT = np.ascontiguousarray(x.T).astype(bf16)
    wp = np.ascontiguousarray(np.asarray(c_proj_w, dtype=np.float32)).astype(bf16)
    identity = np.eye(128, dtype=np.float32).astype(bf16)
    ones = np.ones((128, 1), dtype=np.float32).astype(bf16)
    cbf16 = np.ascontiguousarray(np.concatenate([identity, ones], axis=1))
    # maskT[p, f]: S^T diagonal tile entry (k=p, q=f) masked iff q < k
    p = np.arange(128)
    maskT = np.where(p[None, :] >= p[:, None], 0.0, NEG).astype(np.float32)

    w = np.asarray(c_attn_w, dtype=np.float32)
    bb = np.asarray(c_attn_b, dtype=np.float32)
    in_maps = []
    for i in range(NCORES):
        cols = np.r_[i * 128:(i + 1) * 128]
        wshard = np.concatenate(
            [w[:, cols], w[:, D + cols], w[:, 2 * D + cols]], axis=1)
        bshard = np.stack(
            [bb[cols], bb[D + cols], bb[2 * D + cols]], axis=1)  # [128, 3]
        cf32 = np.ascontiguousarray(
            np.concatenate([maskT, bshard], axis=1)).astype(np.float32)
        in_maps.append({
            "xT": xT,
            "wqkv": np.ascontiguousarray(wshard).astype(bf16),
            "wp": wp,
            "cbf16": cbf16,
            "cf32": cf32,
        })
    return in_maps


def kernel(hidden_states, c_attn_w, c_attn_b, c_proj_w, c_proj_b, _trace=False):
    if "nc" not in _CACHE:
        _CACHE["nc"] = _build()
    nc = _CACHE["nc"]
    in_maps = _prep_inputs(hidden_states, c_attn_w, c_attn_b, c_proj_w)
    try:
        res = run_bass_kernel_spmd(nc, in_maps, core_ids=list(range(NCORES)),
                                   trace=_trace)
    except (ImportError, ModuleNotFoundError):
        # NTFF profiling hook unavailable in this container
        res = run_bass_kernel_spmd(nc, in_maps, core_ids=list(range(NCORES)),
                                   trace=False)
    _CACHE["last_result"] = res
    # core j's output rows: [0:256] = batch0 tokens 256j..;
    # [256:384] = batch1 tokens 128j..; [384:512] = batch1 tokens 1024+128j..
    full = np.empty((NT, D), dtype=np.float32)
    for j in range(NCORES):
        o = res.results[j]["out"]
        full[256 * j:256 * (j + 1)] = o[0:256]
        full[S + 128 * j:S + 128 * (j + 1)] = o[256:384]
        full[S + 1024 + 128 * j:S + 1024 + 128 * (j + 1)] = o[384:512]
    full = full + np.asarray(c_proj_b, dtype=np.float32)[None, :]
    return full.reshape(B, S, D).astype(np.float32)



# revision 16
# speedup vs baseline: 1.2973x; 1.2973x over previous
"""GPT-2 attention block on 8 TRN2 NeuronCores.

Sharding (Megatron-style): core i owns heads (2i, 2i+1) for both batches.
 - QKV projection computed transposed: qkvT = Wshard^T @ X^T  -> [384, 4096]
   (rows: q0|q1|k0|k1|v0|v1 head-dim slices, cols: tokens b-major).
   X^T is DMA'd in token chunks; each chunk's projection is immediately
   followed by that chunk's attention work (b, c = divmod(n, 4)) so the
   scalar/vector/pool engines overlap the PE from the start.
 - scores per (batch, head) in transposed layout S^T[k, q], causal tiles
   only, both heads side by side in one 2-bank PSUM tile; one fused exp on
   ScalarE per tile (1/sqrt(64) folded into the activation); causal mask
   applied as a post-exp 0/1 multiply on the diagonal 128x128 block (DVE,
   16-bit 2x mode).
 - AV flipped: stationary = prob tile [128k x 128q], moving = ones-augmented
   V ([v_h | 1], 65 cols) -> psum [128 q, 65] accumulates av AND the softmax
   denominator per query partition; fully-masked (ki > q-tile) matmuls are
   skipped. One fused tensor_scalar divide normalizes av (per-partition
   denominator scalar straight from PSUM col 64) - no transposes, no
   denominator DRAM bounce.
 - AllToAlls reshard to sequence parallelism; each core runs the output
   projection for its tokens. Batch 0 in one AllToAll (256-token blocks,
   overlaps batch 1); batch 1 in two half-batch AllToAlls (128-token
   blocks) so only the second half sits in the tail.
 - PE p-state warmup: dummy matmuls ramp the tensor engine to full clock
   while the first input DMAs land.
Output per core j: [512, 1024] fp32 - rows 0:256 = batch0 tokens 256j..,
rows 256:384 = batch1 tokens 128j.., rows 384:512 = batch1 tokens
1024+128j..; host reassembles. Matmuls in bf16 (fp32 PSUM accumulation);
softmax in fp32. Post passes: ldweights dedup + splitting multi-wait
instructions into single-wait NoOps (this walrus build caps HW waits at 1).
"""

import numpy as np
import ml_dtypes

import concourse.bass as bass
import concourse.mybir as mybir
import concourse.tile as tile
from concourse.bass_utils import run_bass_kernel_spmd

BF16 = mybir.dt.bfloat16
F32 = mybir.dt.float32
AF = mybir.ActivationFunctionType

B, S, D, H = 2, 2048, 1024, 16
NT = B * S          # 4096 tokens, b-major
NCORES = 8
DK = D // H         # 64
SCALE = 0.125       # 1/sqrt(64)

_CACHE = {}
_NO_COLLECTIVE = False


def _build(debug_dumps=False):
    nc = bass.Bass("TRN2", target_bir_lowering=False, debug=False,
                   num_devices=NCORES)

    xT = nc.dram_tensor("xT", [D, NT], BF16, kind="ExternalInput").ap()
    wqkv = nc.dram_tensor("wqkv", [D, 384], BF16, kind="ExternalInput").ap()
    wp = nc.dram_tensor("wp", [D, D], BF16, kind="ExternalInput").ap()
    cbf16 = nc.dram_tensor("cbf16", [128, 257], BF16, kind="ExternalInput").ap()
    cf32 = nc.dram_tensor("cf32", [128, 3], F32, kind="ExternalInput").ap()
    out = nc.dram_tensor("out", [512, 1024], F32, kind="ExternalOutput").ap()
    dbg = None
    if debug_dumps:
        dbg = {
            "dbg_qkvT": nc.dram_tensor(
                "dbg_qkvT", [128, 3, NT], BF16, kind="ExternalOutput").ap(),
            "dbg_av": nc.dram_tensor(
                "dbg_av", [128, 32, 128], BF16, kind="ExternalOutput").ap(),
            "dbg_aT": nc.dram_tensor(
                "dbg_aT", [128, 8, 512], BF16, kind="ExternalOutput").ap(),
        }

    with tile.TileContext(nc) as tc:
        _body(tc, out, xT, wqkv, wp, cbf16, cf32, dbg)
    _dedup_ldweights(nc)
    _split_multi_waits(nc)
    return nc


def _dedup_ldweights(nc):
    """Drop a back-to-back identical, wait-free Ldweights (weights already
    resident; only Matmults in between; transposes clobber -> reset)."""
    for f in nc.m.functions:
        for bb in f.blocks:
            insts = bb.instructions
            new = []
            changed = False
            last_w = None
            for inst in insts:
                nm = inst.__class__.__name__
                if getattr(inst, "engine", None) == mybir.EngineType.PE:
                    if nm == "InstLdweights":
                        si = inst.sync_info
                        key = repr(inst.ins)
                        no_waits = si is None or not si.on_wait
                        no_upd = si is None or not si.on_update
                        if key == last_w and no_waits and no_upd:
                            changed = True
                            continue  # drop duplicate load
                        last_w = key
                    elif nm == "InstMatmult":
                        if getattr(inst, "is_transpose", False):
                            last_w = None
                    else:
                        last_w = None
                new.append(inst)
            if changed:
                bb.instructions = new


def _split_multi_waits(nc):
    """Walrus caps HW sync waits at 1 per instruction: hoist extras onto
    dedicated NoOps inserted just before the offender (same engine queue)."""
    import bass_rust
    nid = [0]
    for f in nc.m.functions:
        for bb in f.blocks:
            insts = bb.instructions
            new = []
            changed = False
            for inst in insts:
                si = getattr(inst, "sync_info", None)
                if si is not None and len(si.on_wait) > 1:
                    changed = True
                    waits = list(si.on_wait)
                    for w in waits[:-1]:
                        nid[0] += 1
                        nop = mybir.InstNoOp(
                            name=f"I-waitnop-{nid[0]}", ins=[], outs=[])
                        nop.engine = inst.engine
                        nop.sync_info = bass_rust.SyncInfo(
                            on_wait=[w], on_update=[])
                        new.append(nop)
                    inst.sync_info = bass_rust.SyncInfo(
                        on_wait=[waits[-1]], on_update=list(si.on_update))
                new.append(inst)
            if changed:
                bb.instructions = new


def _body(tc, out, xT, wqkv, wp, cbf16, cf32, dbg=None):
    nc = tc.nc

    with (
        tc.tile_pool(name="persist", bufs=1) as persist,
        tc.tile_pool(name="expp", bufs=24) as expp_pool,
        tc.tile_pool(name="smalls", bufs=4) as smalls_pool,
        tc.tile_pool(name="ablk", bufs=2) as ablk_pool,
        tc.tile_pool(name="outs", bufs=3) as outs_pool,
        tc.tile_pool(name="pss", bufs=2, space="PSUM") as pss_pool,
        tc.tile_pool(name="gemm", bufs=2, space="PSUM") as gemm_pool,
        tc.tile_pool(name="avp", bufs=1, space="PSUM") as avp_pool,
        tc.tile_pool(name="tps", bufs=1, space="PSUM") as tps_pool,
        tc.tile_pool(name="dram", bufs=1, space="DRAM") as dram_pool,
    ):
        # ---- persistent SBUF ----
        xT_sb = persist.tile([128, 8, NT], BF16)        # X^T, D-tile major
        wqkv_sb = persist.tile([128, 8, 384], BF16)
        wp_sb = persist.tile([128, 8, 1024], BF16)
        qkvT_sb = persist.tile([128, 3, NT], BF16)      # q|k|v ^T rows
        v_aug = persist.tile([128, 32, 130], BF16)      # [v_h0|1|v_h1|1] per token-tile
        av_sb = persist.tile([128, 32, 128], BF16)      # normalized av, token-major
        aT_sb = persist.tile([128, 8, 512], BF16)       # a^T after all-to-all
        cbf16_sb = persist.tile([128, 257], BF16)
        cf32_sb = persist.tile([128, 3], F32)
        ident_sb = cbf16_sb[:, 0:128]
        maskmul_sb = cbf16_sb[:, 129:257]   # [k, q]: 1.0 if q >= k else 0.0
        bqkv_sb = cf32_sb[:, 0:3]

        av_bounce = {0: dram_pool.tile([S, 128], BF16, name="avb0"),
                     "1a": dram_pool.tile([S // 2, 128], BF16, name="avb1a"),
                     "1b": dram_pool.tile([S // 2, 128], BF16, name="avb1b")}
        recv_bounce = {0: dram_pool.tile([S, 128], BF16, name="rcv0"),
                       "1a": dram_pool.tile([S // 2, 128], BF16, name="rcv1a"),
                       "1b": dram_pool.tile([S // 2, 128], BF16, name="rcv1b")}

        # ones columns of v_aug (rest overwritten by V transposes)
        nc.vector.memset(v_aug[:, :, 64:65], 1.0)
        nc.vector.memset(v_aug[:, :, 129:130], 1.0)

        # ---- ACT warmup: attach table-load pseudos to wait-free instructions
        warm = smalls_pool.tile([1, 2], F32, tag="warm")
        nc.vector.memset(warm[:, 0:1], 0.0)
        nc.scalar.activation(warm[:, 1:2], warm[:, 0:1], AF.Identity)
        nc.scalar.activation(warm[:, 1:2], warm[:, 0:1], AF.Exp)
        nc.scalar.activation(warm[:, 1:2], warm[:, 0:1], AF.Copy)

        # ---- slot-rotated persistent PSUM tiles (bank-granular pool slots
        # would otherwise blow the 8-bank budget)
        avps = avp_pool.tile([128, 7, 65], F32)     # AV psum, 7 slots
        tpss = tps_pool.tile([128, 4, 128], BF16)   # transpose psum, 4 slots
        av_slot = [0]
        tp_slot = [0]

        # ---- PE p-state warmup: ramp the tensor engine to full clock on
        # dummy matmuls while the first input DMAs land (ramp model: full
        # speed after 3us of continuous execution).
        wtile = smalls_pool.tile([128, 64], BF16, tag="wtile")
        nc.vector.memset(wtile[:, :], 0.0)
        wps = gemm_pool.tile([128, 512], F32, tag="gemm")
        for i in range(56):
            nc.tensor.matmul(wps[0:64, 0:64], wtile[:, 0:64], wtile[:, :],
                             start=True, stop=True)

        # ---- input DMAs. First QKV matmuls need wqkv + xT chunk 0; split
        # those by kt-halves so early k-tiles land first.
        nc.sync.dma_start(cbf16_sb[:, :], cbf16[:, :])
        nc.sync.dma_start(cf32_sb[:, :], cf32[:, :])
        wqkv_r = wqkv.rearrange("(kt p) n -> p kt n", p=128)
        nc.sync.dma_start(wqkv_sb[:, 0:4, :], wqkv_r[:, 0:4, :])
        xT0 = xT[:, 0:512].rearrange("(kt p) w -> p kt w", p=128)
        nc.sync.dma_start(xT_sb[:, 0:4, 0:512], xT0[:, 0:4, :])
        nc.sync.dma_start(wqkv_sb[:, 4:8, :], wqkv_r[:, 4:8, :])
        nc.sync.dma_start(xT_sb[:, 4:8, 0:512], xT0[:, 4:8, :])
        for n in range(1, 8):
            nc.sync.dma_start(
                xT_sb[:, :, n * 512:(n + 1) * 512],
                xT[:, n * 512:(n + 1) * 512]
                .rearrange("(kt p) w -> p kt w", p=128))
        nc.sync.dma_start(wp_sb[:, :, :],
                          wp.rearrange("(kt p) n -> p kt n", p=128))

        def qkv_chunk(n):
            # qkvT[:, :, n*512:(n+1)*512] = Wshard^T @ X^T chunk + bias
            for m in range(3):
                ps = gemm_pool.tile([128, 512], F32, tag="gemm")
                for kt in range(8):
                    nc.tensor.matmul(
                        ps[:, :],
                        wqkv_sb[:, kt, m * 128:(m + 1) * 128],
                        xT_sb[:, kt, n * 512:(n + 1) * 512],
                        start=(kt == 0), stop=(kt == 7),
                    )
                nc.vector.tensor_scalar_add(
                    qkvT_sb[:, m, n * 512:(n + 1) * 512],
                    ps[:, :], bqkv_sb[:, m:m + 1])
            # V transposes -> natural layout, ones-augmented
            for t in range(n * 4, n * 4 + 4):
                sl = tp_slot[0] % 4
                tp_slot[0] += 1
                ps_t = tpss[:, sl, :]
                nc.tensor.transpose(
                    ps_t, qkvT_sb[:, 2, t * 128:(t + 1) * 128],
                    ident_sb[:, :])
                # both head blocks in one strided copy (ones col at 64 kept)
                nc.vector.tensor_copy(
                    v_aug[:, t:t + 1, 0:130]
                    .rearrange("p a (h q) -> p (a h) q", h=2)[:, :, 0:64],
                    ps_t.rearrange("p (h q) -> p h q", h=2))

        def attn_chunk(b, c):
            tok0 = b * S
            q0 = tok0 + c * 512
            nk = 4 * c + 4
            ets = []
            # scores (S^T layout) + exp per k-tile; post-exp causal multiply
            for ki in range(nk):
                off = max(0, (ki - 4 * c)) * 128
                w = 512 - off
                ps_s = pss_pool.tile([128, 2, 512], F32, tag="pss")
                for h in range(2):
                    hp = h * 64
                    nc.tensor.matmul(
                        ps_s[:, h, :w],
                        qkvT_sb[hp:hp + 64, 1,
                                tok0 + ki * 128: tok0 + (ki + 1) * 128],
                        qkvT_sb[hp:hp + 64, 0, q0 + off: q0 + 512],
                        start=True, stop=True,
                    )
                et = expp_pool.tile([128, 2, 512], BF16, tag="expp")
                nc.scalar.activation(
                    et[:, :, :w], ps_s[:, :, :w], AF.Exp, scale=SCALE)
                if ki >= 4 * c:  # diagonal tile: zero masked (q < k) probs
                    for h in range(2):
                        nc.gpsimd.tensor_mul(
                            et[:, h, 0:128], et[:, h, 0:128], maskmul_sb)
                ets.append((et, off))
            # flipped AV: stationary = prob tile, moving = [v_h | 1];
            # psum col 64 = softmax denominator per query partition.
            for h in range(2):
                for qt in range(4):
                    gq = 4 * c + qt
                    sl = av_slot[0] % 7
                    av_slot[0] += 1
                    ps_av = avps[:, sl, :]
                    for ki in range(gq + 1):
                        et, off = ets[ki]
                        col0 = qt * 128 - off
                        nc.tensor.matmul(
                            ps_av,
                            et[:, h, col0:col0 + 128],
                            v_aug[:, b * 16 + ki, h * 65:(h + 1) * 65],
                            start=(ki == 0), stop=(ki == gq),
                        )
                    rec = smalls_pool.tile([128, 1], F32, tag="rec")
                    nc.vector.reciprocal(rec[:, :], ps_av[:, 64:65])
                    nc.vector.tensor_scalar_mul(
                        av_sb[:, b * 16 + gq, h * 64:(h + 1) * 64],
                        ps_av[:, 0:64], rec[:, :])

        def a2a(key, t0, nt):
            # all-to-all over av token-tiles [t0, t0+nt): nt*16-token blocks
            nc.sync.dma_start(
                av_bounce[key].rearrange("(t p) d -> p t d", p=128),
                av_sb[:, t0:t0 + nt, :])
            if _NO_COLLECTIVE:
                nc.sync.dma_start(recv_bounce[key][:, :], av_bounce[key][:, :])
            else:
                nc.gpsimd.collective_compute(
                    "AllToAll", mybir.AluOpType.bypass,
                    replica_groups=[list(range(NCORES))],
                    ins=[av_bounce[key][:, :].opt()],
                    outs=[recv_bounce[key][:, :].opt()],
                )

        def recv_stage(key, jn, col0):
            # rebuild a^T [dcol, jn*128 tokens] at aT_sb cols [col0, ...)
            a_stg = ablk_pool.tile([128, 8 * jn, 128], BF16, tag="ablk",
                                   name=f"astg{key}")
            nc.sync.dma_start(
                a_stg[:, :, :],
                recv_bounce[key].rearrange("(t p) d -> p t d", p=128))
            for s in range(8):
                for j in range(jn):
                    sl = tp_slot[0] % 4
                    tp_slot[0] += 1
                    ps_t = tpss[:, sl, :]
                    nc.tensor.transpose(ps_t, a_stg[:, s * jn + j, :],
                                        ident_sb[:, :])
                    nc.vector.tensor_copy(
                        aT_sb[:, s, col0 + j * 128: col0 + (j + 1) * 128],
                        ps_t)

        def proj(b, mts=(0, 1)):
            for mt in mts:
                r0 = b * 256 + mt * 128
                o_sb = outs_pool.tile([128, 1024], F32, tag="outs")
                for n2 in range(2):
                    ps = gemm_pool.tile([128, 512], F32, tag="gemm")
                    for s in range(8):
                        nc.tensor.matmul(
                            ps[:, :],
                            aT_sb[:, s, r0:r0 + 128],
                            wp_sb[:, s, n2 * 512:(n2 + 1) * 512],
                            start=(s == 0), stop=(s == 7),
                        )
                    nc.vector.tensor_copy(
                        o_sb[:, n2 * 512:(n2 + 1) * 512], ps[:, :])
                nc.sync.dma_start(out[r0:r0 + 128, :], o_sb[:, :])

        # ---- main pipeline: QKV chunk n feeds attention chunk (b, c) ----
        for n in range(8):
            qkv_chunk(n)
            b, c = divmod(n, 4)
            attn_chunk(b, c)
            if (b, c) == (0, 3):
                a2a(0, 0, 16)
            elif (b, c) == (1, 1):
                a2a("1a", 16, 8)
            elif (b, c) == (1, 2):
                recv_stage(0, 2, 0)
                proj(0)
        a2a("1b", 24, 8)
        recv_stage("1a", 1, 256)
        proj(1, mts=(0,))
        recv_stage("1b", 1, 384)
        proj(1, mts=(1,))

        if dbg is not None:
            nc.sync.dma_start(
                dbg["dbg_qkvT"].rearrange("p m n -> p (m n)"),
                qkvT_sb[:, :, :].rearrange("p m n -> p (m n)"))
            nc.sync.dma_start(
                dbg["dbg_av"].rearrange("p t d -> p (t d)"),
                av_sb[:, :, :].rearrange("p t d -> p (t d)"))
            nc.sync.dma_start(
                dbg["dbg_aT"].rearrange("p s n -> p (s n)"),
                aT_sb[:, :, :].rearrange("p s n -> p (s n)"))


def _prep_inputs(hidden_states, c_attn_w, c_attn_b, c_proj_w):
    bf16 = ml_dtypes.bfloat16
    x = np.asarray(hidden_states, dtype=np.float32).reshape(NT, D)
    xT = np.ascontiguousarray(x.T).astype(bf16)
    wp = np.ascontiguousarray(np.asarray(c_proj_w, dtype=np.float32)).astype(bf16)
    identity = np.eye(128, dtype=np.float32)
    ones = np.ones((128, 1), dtype=np.float32)
    # maskmul[k, q] (S^T diagonal tile): keep iff q >= k
    p = np.arange(128)
    maskmul = (p[None, :] >= p[:, None]).astype(np.float32)
    cbf16 = np.ascontiguousarray(
        np.concatenate([identity, ones, maskmul], axis=1)).astype(bf16)

    w = np.asarray(c_attn_w, dtype=np.float32)
    bb = np.asarray(c_attn_b, dtype=np.float32)
    in_maps = []
    for i in range(NCORES):
        cols = np.r_[i * 128:(i + 1) * 128]
        wshard = np.concatenate(
            [w[:, cols], w[:, D + cols], w[:, 2 * D + cols]], axis=1)
        bshard = np.stack(
            [bb[cols], bb[D + cols], bb[2 * D + cols]], axis=1)  # [128, 3]
        cf32 = np.ascontiguousarray(bshard).astype(np.float32)
        in_maps.append({
            "xT": xT,
            "wqkv": np.ascontiguousarray(wshard).astype(bf16),
            "wp": wp,
            "cbf16": cbf16,
            "cf32": cf32,
        })
    return in_maps


def kernel(hidden_states, c_attn_w, c_attn_b, c_proj_w, c_proj_b, _trace=False):
    if "nc" not in _CACHE:
        _CACHE["nc"] = _build()
    nc = _CACHE["nc"]
    in_maps = _prep_inputs(hidden_states, c_attn_w, c_attn_b, c_proj_w)
    try:
        res = run_bass_kernel_spmd(nc, in_maps, core_ids=list(range(NCORES)),
                                   trace=_trace)
    except (ImportError, ModuleNotFoundError):
        # NTFF profiling hook unavailable in this container
        res = run_bass_kernel_spmd(nc, in_maps, core_ids=list(range(NCORES)),
                                   trace=False)
    _CACHE["last_result"] = res
    # core j's output rows: [0:256] = batch0 tokens 256j..;
    # [256:384] = batch1 tokens 128j..; [384:512] = batch1 tokens 1024+128j..
    full = np.empty((NT, D), dtype=np.float32)
    for j in range(NCORES):
        o = res.results[j]["out"]
        full[256 * j:256 * (j + 1)] = o[0:256]
        full[S + 128 * j:S + 128 * (j + 1)] = o[256:384]
        full[S + 1024 + 128 * j:S + 1024 + 128 * (j + 1)] = o[384:512]
    full = full + np.asarray(c_proj_b, dtype=np.float32)[None, :]
    return full.reshape(B, S, D).astype(np.float32)


# revision 46
# speedup vs baseline: 1.3909x; 1.0722x over previous
"""GPT-2 attention block on 8 TRN2 NeuronCores.

Sharding (Megatron-style): core i owns heads (2i, 2i+1) for both batches.
 - QKV projection computed transposed: qkvT = Wshard^T @ X^T  -> [384, 4096]
   (rows: q0|q1|k0|k1|v0|v1 head-dim slices, cols: tokens b-major).
   X^T is DMA'd in token chunks; each chunk's projection is immediately
   followed by that chunk's attention work (b, c = divmod(n, 4)) so the
   scalar/vector/pool engines overlap the PE from the start.
 - scores per (batch, head) in transposed layout S^T[k, q], causal tiles
   only, both heads side by side in one 2-bank PSUM tile; one fused exp on
   ScalarE per tile (1/sqrt(64) folded into the activation); causal mask
   applied as a post-exp 0/1 multiply on the diagonal 128x128 block (DVE,
   16-bit 2x mode).
 - AV flipped: stationary = prob tile [128k x 128q], moving = ones-augmented
   V ([v_h | 1], 65 cols) -> psum [128 q, 65] accumulates av AND the softmax
   denominator per query partition; fully-masked (ki > q-tile) matmuls are
   skipped. One fused tensor_scalar divide normalizes av (per-partition
   denominator scalar straight from PSUM col 64) - no transposes, no
   denominator DRAM bounce.
 - AllToAlls reshard to sequence parallelism; each core runs the output
   projection for its tokens. Batch 0 in one AllToAll (256-token blocks,
   overlaps batch 1); batch 1 in two half-batch AllToAlls (128-token
   blocks) so only the second half sits in the tail.
 - PE p-state warmup: dummy matmuls ramp the tensor engine to full clock
   while the first input DMAs land.
Output per core j: [512, 1024] fp32 - rows 0:256 = batch0 tokens 256j..,
rows 256:384 = batch1 tokens 128j.., rows 384:512 = batch1 tokens
1024+128j..; host reassembles. Matmuls in bf16 (fp32 PSUM accumulation);
softmax in fp32. Post passes: ldweights dedup + splitting multi-wait
instructions into single-wait NoOps (this walrus build caps HW waits at 1).
"""

import numpy as np
import ml_dtypes

import concourse.bass as bass
import concourse.mybir as mybir
import concourse.tile as tile
from concourse.bass_utils import run_bass_kernel_spmd

BF16 = mybir.dt.bfloat16
F32 = mybir.dt.float32
AF = mybir.ActivationFunctionType

B, S, D, H = 2, 2048, 1024, 16
NT = B * S          # 4096 tokens, b-major
NCORES = 8
DK = D // H         # 64
SCALE = 0.125       # 1/sqrt(64)

_CACHE = {}
_NO_COLLECTIVE = False


def _build(debug_dumps=False):
    nc = bass.Bass("TRN2", target_bir_lowering=False, debug=False,
                   num_devices=NCORES)

    xT = nc.dram_tensor("xT", [D, NT], BF16, kind="ExternalInput").ap()
    wqkv = nc.dram_tensor("wqkv", [D, 384], BF16, kind="ExternalInput").ap()
    wp = nc.dram_tensor("wp", [D, D], BF16, kind="ExternalInput").ap()
    cbf16 = nc.dram_tensor("cbf16", [128, 257], BF16, kind="ExternalInput").ap()
    cf32 = nc.dram_tensor("cf32", [128, 3], F32, kind="ExternalInput").ap()
    out = nc.dram_tensor("out", [512, 1024], BF16, kind="ExternalOutput").ap()
    dbg = None
    if debug_dumps:
        dbg = {
            "dbg_qkvT": nc.dram_tensor(
                "dbg_qkvT", [128, 3, NT], BF16, kind="ExternalOutput").ap(),
            "dbg_av": nc.dram_tensor(
                "dbg_av", [128, 32, 128], BF16, kind="ExternalOutput").ap(),
            "dbg_aT": nc.dram_tensor(
                "dbg_aT", [128, 8, 512], BF16, kind="ExternalOutput").ap(),
        }

    with tile.TileContext(nc) as tc:
        _body(tc, out, xT, wqkv, wp, cbf16, cf32, dbg)
    _dedup_ldweights(nc)
    _split_multi_waits(nc)
    return nc


def _dedup_ldweights(nc):
    """Drop a back-to-back identical, wait-free Ldweights (weights already
    resident; only Matmults in between; transposes clobber -> reset)."""
    for f in nc.m.functions:
        for bb in f.blocks:
            insts = bb.instructions
            new = []
            changed = False
            last_w = None
            for inst in insts:
                nm = inst.__class__.__name__
                if getattr(inst, "engine", None) == mybir.EngineType.PE:
                    if nm == "InstLdweights":
                        si = inst.sync_info
                        key = repr(inst.ins)
                        no_waits = si is None or not si.on_wait
                        no_upd = si is None or not si.on_update
                        if key == last_w and no_waits and no_upd:
                            changed = True
                            continue  # drop duplicate load
                        last_w = key
                    elif nm == "InstMatmult":
                        if getattr(inst, "is_transpose", False):
                            last_w = None
                    else:
                        last_w = None
                new.append(inst)
            if changed:
                bb.instructions = new


def _split_multi_waits(nc):
    """Walrus caps HW sync waits at 1 per instruction: hoist extras onto
    dedicated NoOps inserted just before the offender (same engine queue)."""
    import bass_rust
    nid = [0]
    for f in nc.m.functions:
        for bb in f.blocks:
            insts = bb.instructions
            new = []
            changed = False
            for inst in insts:
                si = getattr(inst, "sync_info", None)
                if si is not None and len(si.on_wait) > 1:
                    changed = True
                    waits = list(si.on_wait)
                    for w in waits[:-1]:
                        nid[0] += 1
                        nop = mybir.InstNoOp(
                            name=f"I-waitnop-{nid[0]}", ins=[], outs=[])
                        nop.engine = inst.engine
                        nop.sync_info = bass_rust.SyncInfo(
                            on_wait=[w], on_update=[])
                        new.append(nop)
                    inst.sync_info = bass_rust.SyncInfo(
                        on_wait=[waits[-1]], on_update=list(si.on_update))
                new.append(inst)
            if changed:
                bb.instructions = new


def _body(tc, out, xT, wqkv, wp, cbf16, cf32, dbg=None):
    nc = tc.nc

    with (
        tc.tile_pool(name="persist", bufs=1) as persist,
        tc.tile_pool(name="expp", bufs=24) as expp_pool,
        tc.tile_pool(name="smalls", bufs=4) as smalls_pool,
        tc.tile_pool(name="ablk", bufs=1) as ablk_pool,
        tc.tile_pool(name="outs", bufs=3) as outs_pool,
        tc.tile_pool(name="pss", bufs=2, space="PSUM") as pss_pool,
        tc.tile_pool(name="gemm", bufs=2, space="PSUM") as gemm_pool,
        tc.tile_pool(name="avp", bufs=1, space="PSUM") as avp_pool,
        tc.tile_pool(name="tps", bufs=1, space="PSUM") as tps_pool,
        tc.tile_pool(name="dram", bufs=1, space="DRAM") as dram_pool,
    ):
        # ---- persistent SBUF ----
        xT_sb = persist.tile([128, 8, NT], BF16)        # X^T, D-tile major
        wqkv_sb = persist.tile([128, 8, 384], BF16)
        wp_sb = persist.tile([128, 8, 1024], BF16)
        qkvT_sb = persist.tile([128, 3, NT], BF16)      # q|k|v ^T rows
        v_aug = persist.tile([128, 32, 130], BF16)      # [v_h0|1|v_h1|1] per token-tile
        av_sb = persist.tile([128, 32, 128], BF16)      # normalized av, token-major
        aT_sb = persist.tile([128, 8, 512], BF16)       # a^T after all-to-all
        cbf16_sb = persist.tile([128, 257], BF16)
        cf32_sb = persist.tile([128, 3], F32)
        ident_sb = cbf16_sb[:, 0:128]
        maskmul_sb = cbf16_sb[:, 129:257]   # [k, q]: 1.0 if q >= k else 0.0
        bqkv_sb = cf32_sb[:, 0:3]

        av_bounce = {0: dram_pool.tile([S, 128], BF16, name="avb0"),
                     "1a": dram_pool.tile([S // 2, 128], BF16, name="avb1a"),
                     "1b": dram_pool.tile([S // 4, 128], BF16, name="avb1b"),
                     "1c": dram_pool.tile([S // 4, 128], BF16, name="avb1c")}
        recv_bounce = {0: dram_pool.tile([S, 128], BF16, name="rcv0"),
                       "1a": dram_pool.tile([S // 2, 128], BF16, name="rcv1a"),
                       "1b": dram_pool.tile([S // 4, 128], BF16, name="rcv1b"),
                       "1c": dram_pool.tile([S // 4, 128], BF16, name="rcv1c")}

        # ones columns of v_aug (rest overwritten by V transposes)
        nc.vector.memset(v_aug[:, :, 64:65], 1.0)
        nc.vector.memset(v_aug[:, :, 129:130], 1.0)

        # ---- ACT warmup: attach table-load pseudos to wait-free instructions
        warm = smalls_pool.tile([1, 2], F32, tag="warm")
        nc.vector.memset(warm[:, 0:1], 0.0)
        nc.scalar.activation(warm[:, 1:2], warm[:, 0:1], AF.Identity)
        nc.scalar.activation(warm[:, 1:2], warm[:, 0:1], AF.Exp)
        nc.scalar.activation(warm[:, 1:2], warm[:, 0:1], AF.Copy)

        # ---- slot-rotated persistent PSUM tiles (bank-granular pool slots
        # would otherwise blow the 8-bank budget)
        avps = avp_pool.tile([128, 7, 65], F32)     # AV psum, 7 slots
        # transpose psum, 4 slots + slot 4 reserved for warm-keeping dummies
        tpss = tps_pool.tile([128, 5, 128], BF16)
        wdum = tpss[0:64, 4, :]
        av_slot = [0]
        tp_slot = [0]

        # ---- PE p-state warmup: ramp the tensor engine to full clock on
        # dummy matmuls while the first input DMAs land (ramp model: full
        # speed after 3us of continuous execution).
        wtile = smalls_pool.tile([128, 64], BF16, tag="wtile")
        nc.vector.memset(wtile[:, :], 0.0)
        wps = gemm_pool.tile([128, 512], F32, tag="gemm")
        for i in range(72):
            nc.tensor.matmul(wps[0:64, 0:64], wtile[:, 0:64], wtile[:, :],
                             start=True, stop=True)

        # ---- input DMAs. First QKV matmuls need wqkv + xT chunk 0; split
        # those by kt-pairs so early k-tiles land first.
        wqkv_r = wqkv.rearrange("(kt p) n -> p kt n", p=128)
        xT0 = xT[:, 0:512].rearrange("(kt p) w -> p kt w", p=128)
        nc.sync.dma_start(wqkv_sb[:, 0:2, :], wqkv_r[:, 0:2, :])
        nc.sync.dma_start(xT_sb[:, 0:2, 0:512], xT0[:, 0:2, :])
        nc.sync.dma_start(cf32_sb[:, :], cf32[:, :])
        nc.sync.dma_start(cbf16_sb[:, :], cbf16[:, :])
        for kt in range(2, 8, 2):
            nc.sync.dma_start(wqkv_sb[:, kt:kt + 2, :], wqkv_r[:, kt:kt + 2, :])
            nc.sync.dma_start(xT_sb[:, kt:kt + 2, 0:512], xT0[:, kt:kt + 2, :])
        for n in range(1, 8):
            nc.sync.dma_start(
                xT_sb[:, :, n * 512:(n + 1) * 512],
                xT[:, n * 512:(n + 1) * 512]
                .rearrange("(kt p) w -> p kt w", p=128))
        nc.sync.dma_start(wp_sb[:, :, :],
                          wp.rearrange("(kt p) n -> p kt n", p=128))

        def qkv_chunk(n):
            # qkvT[:, :, n*512:(n+1)*512] = Wshard^T @ X^T chunk + bias
            for m in range(3):
                ps = gemm_pool.tile([128, 512], F32, tag="gemm")
                for kt in range(8):
                    nc.tensor.matmul(
                        ps[:, :],
                        wqkv_sb[:, kt, m * 128:(m + 1) * 128],
                        xT_sb[:, kt, n * 512:(n + 1) * 512],
                        start=(kt == 0), stop=(kt == 7),
                    )
                nc.vector.tensor_scalar_add(
                    qkvT_sb[:, m, n * 512:(n + 1) * 512],
                    ps[:, :], bqkv_sb[:, m:m + 1])
            # V transposes -> natural layout, ones-augmented. All four
            # transposes before the copies (bank-granular PSUM deps).
            pts = []
            for t in range(n * 4, n * 4 + 4):
                sl = tp_slot[0] % 4
                tp_slot[0] += 1
                ps_t = tpss[:, sl, :]
                nc.tensor.transpose(
                    ps_t, qkvT_sb[:, 2, t * 128:(t + 1) * 128],
                    ident_sb[:, :])
                pts.append(ps_t)
            for t, ps_t in zip(range(n * 4, n * 4 + 4), pts):
                # both head blocks in one strided copy (ones col at 64 kept)
                nc.vector.tensor_copy(
                    v_aug[:, t:t + 1, 0:130]
                    .rearrange("p a (h q) -> p (a h) q", h=2)[:, :, 0:64],
                    ps_t.rearrange("p (h q) -> p h q", h=2))

        def attn_chunk(b, c):
            tok0 = b * S
            q0 = tok0 + c * 512
            nk = 4 * c + 4
            ets = []

            def av_matmuls(h, qt):
                # flipped AV: stationary = prob tile, moving = [v_h | 1];
                # psum col 64 = softmax denominator per query partition.
                gq = 4 * c + qt
                sl = av_slot[0] % 7
                av_slot[0] += 1
                ps_av = avps[:, sl, :]
                for ki in range(gq + 1):
                    et, off = ets[ki]
                    col0 = qt * 128 - off
                    nc.tensor.matmul(
                        ps_av,
                        et[:, h, col0:col0 + 128],
                        v_aug[:, b * 16 + ki, h * 65:(h + 1) * 65],
                        start=(ki == 0), stop=(ki == gq),
                    )
                return ps_av

            def av_normalize(ps_av, h, qt):
                gq = 4 * c + qt
                rec = smalls_pool.tile([128, 1], F32, tag="rec")
                nc.vector.reciprocal(rec[:, :], ps_av[:, 64:65])
                nc.vector.tensor_scalar_mul(
                    av_sb[:, b * 16 + gq, h * 64:(h + 1) * 64],
                    ps_av[:, 0:64], rec[:, :])

            # scores (S^T layout) + exp per k-tile; post-exp causal multiply.
            # AV group (h, qt) is emitted as soon as its last prob tile
            # (diagonal ki = 4c + qt) exists, so only the last groups wait
            # on the final exp of the chunk.
            for ki in range(nk):
                off = max(0, (ki - 4 * c)) * 128
                w = 512 - off
                ps_s = pss_pool.tile([128, 2, 512], F32, tag="pss")
                for h in range(2):
                    hp = h * 64
                    nc.tensor.matmul(
                        ps_s[:, h, :w],
                        qkvT_sb[hp:hp + 64, 1,
                                tok0 + ki * 128: tok0 + (ki + 1) * 128],
                        qkvT_sb[hp:hp + 64, 0, q0 + off: q0 + 512],
                        start=True, stop=True,
                    )
                et = expp_pool.tile([128, 2, 512], BF16, tag="expp")
                nc.scalar.activation(
                    et[:, :, :w], ps_s[:, :, :w], AF.Exp, scale=SCALE)
                if ki >= 4 * c:  # diagonal tile: zero masked (q < k) probs
                    # one head per engine so both mults run in parallel
                    nc.vector.tensor_mul(
                        et[:, 0, 0:128], et[:, 0, 0:128], maskmul_sb)
                    nc.gpsimd.tensor_mul(
                        et[:, 1, 0:128], et[:, 1, 0:128], maskmul_sb)
                ets.append((et, off))
                if ki >= 4 * c:
                    # both heads' AV matmul groups before either normalize:
                    # PSUM dep tracking is bank-granular, so a group's first
                    # (start=True) write would otherwise stall on the
                    # previous group's normalize reads of the shared bank.
                    qt = ki - 4 * c
                    ps0 = av_matmuls(0, qt)
                    ps1 = av_matmuls(1, qt)
                    av_normalize(ps0, 0, qt)
                    av_normalize(ps1, 1, qt)

        def a2a(key, t0, nt):
            # all-to-all over av token-tiles [t0, t0+nt), p-major rows
            # (row = p*nt + t~): block j <=> partitions [16j, 16j+16), i.e.
            # core j receives tokens with t%128 in [16j, 16j+16). p-major
            # keeps both bounce DMAs fully contiguous per partition.
            nc.sync.dma_start(
                av_bounce[key].rearrange("(p t) d -> p t d", t=nt),
                av_sb[:, t0:t0 + nt, :])
            if _NO_COLLECTIVE:
                nc.sync.dma_start(recv_bounce[key][:, :], av_bounce[key][:, :])
            else:
                nc.gpsimd.collective_compute(
                    "AllToAll", mybir.AluOpType.bypass,
                    replica_groups=[list(range(NCORES))],
                    ins=[av_bounce[key][:, :].opt()],
                    outs=[recv_bounce[key][:, :].opt()],
                )

        a_stgs = {}

        def recv_dma(key, nt):
            # issue the recv read as soon as the collective output exists so
            # it never queues behind a later, still-blocked bounce DMA
            a_stg = ablk_pool.tile([128, nt, 128], BF16, tag=f"ablk{key}",
                                   name=f"astg{key}")
            nc.sync.dma_start(
                a_stg[:, :, :],
                recv_bounce[key].rearrange("(k p) d -> p k d", p=128))
            a_stgs[key] = a_stg

        def recv_work(key, nt, col0):
            # rebuild a^T at aT_sb cols [col0, col0 + nt*128/spt). src block
            # = nt*16 rows; a_stg tile k spans 128 rows.
            a_stg = a_stgs[key]
            for k0 in range(0, nt, 4):     # groups of 4 = tpss slot count
                pts = []
                for k in range(k0, min(k0 + 4, nt)):
                    sl = tp_slot[0] % 4
                    tp_slot[0] += 1
                    ps_t = tpss[:, sl, :]
                    nc.tensor.transpose(ps_t, a_stg[:, k, :], ident_sb[:, :])
                    pts.append((k, ps_t))
                for k, ps_t in pts:
                    if nt >= 8:
                        spt = nt // 8      # a_stg tiles per src block
                        s, j = k // spt, k % spt
                        nc.vector.tensor_copy(
                            aT_sb[:, s, col0 + j * 128: col0 + (j + 1) * 128],
                            ps_t)
                    else:                  # two 64-row src blocks per tile
                        nc.vector.tensor_copy(
                            aT_sb[:, 2 * k, col0:col0 + 64], ps_t[:, 0:64])
                        nc.vector.tensor_copy(
                            aT_sb[:, 2 * k + 1, col0:col0 + 64],
                            ps_t[:, 64:128])

        def proj(b, mts=(0, 1)):
            for mt in mts:
                r0 = b * 256 + mt * 128
                for n2 in range(2):
                    ps = gemm_pool.tile([128, 512], F32, tag="gemm")
                    for s in range(8):
                        nc.tensor.matmul(
                            ps[:, :],
                            aT_sb[:, s, r0:r0 + 128],
                            wp_sb[:, s, n2 * 512:(n2 + 1) * 512],
                            start=(s == 0), stop=(s == 7),
                        )
                    o_sb = outs_pool.tile([128, 512], BF16, tag="outs")
                    nc.vector.tensor_copy(o_sb[:, :], ps[:, :])
                    nc.sync.dma_start(
                        out[r0:r0 + 128, n2 * 512:(n2 + 1) * 512], o_sb[:, :])

        # ---- main pipeline: QKV chunk n feeds attention chunk (b, c) ----
        for n in range(8):
            qkv_chunk(n)
            b, c = divmod(n, 4)
            attn_chunk(b, c)
        # tile_wait_until stamps pin the scheduler's SP-queue order for the
        # a2a chains (its internal sim otherwise reorders them and the
        # DMA-completion counters then serialize every chain into the tail).
            if (b, c) == (0, 3):
                a2a(0, 0, 16)
                recv_dma(0, 16)
            elif (b, c) == (1, 1):
                with tc.tile_wait_until(0.100):
                    a2a("1a", 16, 8)
                    recv_dma("1a", 8)
            elif (b, c) == (1, 2):
                a2a("1b", 24, 4)       # chunk (1,2): chain hides under (1,3)
                recv_dma("1b", 4)
                recv_work(0, 16, 0)
                proj(0)
        a2a("1c", 28, 4)
        recv_dma("1c", 4)
        recv_work("1a", 8, 256)
        proj(1, mts=(0,))
        recv_work("1b", 4, 384)
        # keep the PE clock ramped through the 1c bounce->collective->read
        # chain (~7us): an idle tensor engine drops to the slow p-state and
        # the tail proj would then run 2-4x slow for its first 3us. Dummy
        # transposes target a dedicated psum region nothing reads.
        for i in range(130):
            nc.tensor.transpose(wdum[:, :], wtile[:, :], ident_sb[:, :])
        recv_work("1c", 4, 448)
        proj(1, mts=(1,))

        if dbg is not None:
            nc.sync.dma_start(
                dbg["dbg_qkvT"].rearrange("p m n -> p (m n)"),
                qkvT_sb[:, :, :].rearrange("p m n -> p (m n)"))
            nc.sync.dma_start(
                dbg["dbg_av"].rearrange("p t d -> p (t d)"),
                av_sb[:, :, :].rearrange("p t d -> p (t d)"))
            nc.sync.dma_start(
                dbg["dbg_aT"].rearrange("p s n -> p (s n)"),
                aT_sb[:, :, :].rearrange("p s n -> p (s n)"))


def _prep_inputs(hidden_states, c_attn_w, c_attn_b, c_proj_w):
    bf16 = ml_dtypes.bfloat16
    x = np.asarray(hidden_states, dtype=np.float32).reshape(NT, D)
    xT = np.ascontiguousarray(x.T).astype(bf16)
    wp = np.ascontiguousarray(np.asarray(c_proj_w, dtype=np.float32)).astype(bf16)
    identity = np.eye(128, dtype=np.float32)
    ones = np.ones((128, 1), dtype=np.float32)
    # maskmul[k, q] (S^T diagonal tile): keep iff q >= k
    p = np.arange(128)
    maskmul = (p[None, :] >= p[:, None]).astype(np.float32)
    cbf16 = np.ascontiguousarray(
        np.concatenate([identity, ones, maskmul], axis=1)).astype(bf16)

    w = np.asarray(c_attn_w, dtype=np.float32)
    bb = np.asarray(c_attn_b, dtype=np.float32)
    in_maps = []
    for i in range(NCORES):
        cols = np.r_[i * 128:(i + 1) * 128]
        wshard = np.concatenate(
            [w[:, cols], w[:, D + cols], w[:, 2 * D + cols]], axis=1)
        bshard = np.stack(
            [bb[cols], bb[D + cols], bb[2 * D + cols]], axis=1)  # [128, 3]
        cf32 = np.ascontiguousarray(bshard).astype(np.float32)
        in_maps.append({
            "xT": xT,
            "wqkv": np.ascontiguousarray(wshard).astype(bf16),
            "wp": wp,
            "cbf16": cbf16,
            "cf32": cf32,
        })
    return in_maps


def kernel(hidden_states, c_attn_w, c_attn_b, c_proj_w, c_proj_b, _trace=False):
    if "nc" not in _CACHE:
        _CACHE["nc"] = _build()
    nc = _CACHE["nc"]
    in_maps = _prep_inputs(hidden_states, c_attn_w, c_attn_b, c_proj_w)
    try:
        res = run_bass_kernel_spmd(nc, in_maps, core_ids=list(range(NCORES)),
                                   trace=_trace)
    except (ImportError, ModuleNotFoundError):
        # NTFF profiling hook unavailable in this container
        res = run_bass_kernel_spmd(nc, in_maps, core_ids=list(range(NCORES)),
                                   trace=False)
    _CACHE["last_result"] = res
    # p-major a2a blocks: core j owns tokens with t%128 in [16j, 16j+16).
    # row idx within each segment decodes as (pi, t~): token = t~*128+16j+pi.
    idx0 = np.arange(256)
    tok0 = (idx0 % 16) * 128 + (idx0 // 16)        # b0 segment (nt=16)
    idx1 = np.arange(128)
    tok1 = (idx1 % 8) * 128 + (idx1 // 8)          # b1 first half (nt=8)
    idx2 = np.arange(64)
    tok2 = (idx2 % 4) * 128 + (idx2 // 4)          # b1 quarter segs (nt=4)
    full = np.empty((NT, D), dtype=np.float32)
    for j in range(NCORES):
        o = np.asarray(res.results[j]["out"], dtype=np.float32)
        full[tok0 + 16 * j] = o[0:256]
        full[S + tok1 + 16 * j] = o[256:384]
        full[S + 1024 + tok2 + 16 * j] = o[384:448]
        full[S + 1536 + tok2 + 16 * j] = o[448:512]
    full = full + np.asarray(c_proj_b, dtype=np.float32)[None, :]
    return full.reshape(B, S, D).astype(np.float32)


# revision 58
# speedup vs baseline: 1.4230x; 1.0231x over previous
"""GPT-2 attention block on 8 TRN2 NeuronCores.

Sharding (Megatron-style): core i owns heads (2i, 2i+1) for both batches.
 - QKV projection computed transposed: qkvT = Wshard^T @ X^T  -> [384, 4096]
   (rows: q0|q1|k0|k1|v0|v1 head-dim slices, cols: tokens b-major).
   X^T is DMA'd in token chunks; each chunk's projection is immediately
   followed by that chunk's attention work (b, c = divmod(n, 4)) so the
   scalar/vector/pool engines overlap the PE from the start.
 - scores per (batch, head) in transposed layout S^T[k, q], causal tiles
   only, both heads side by side in one 2-bank PSUM tile; one fused exp on
   ScalarE per tile (1/sqrt(64) folded into the activation); causal mask
   applied as a post-exp 0/1 multiply on the diagonal 128x128 block (DVE,
   16-bit 2x mode).
 - AV flipped: stationary = prob tile [128k x 128q], moving = ones-augmented
   V ([v_h | 1], 65 cols) -> psum [128 q, 65] accumulates av AND the softmax
   denominator per query partition; fully-masked (ki > q-tile) matmuls are
   skipped. One fused tensor_scalar divide normalizes av (per-partition
   denominator scalar straight from PSUM col 64) - no transposes, no
   denominator DRAM bounce.
 - AllToAlls reshard to sequence parallelism; each core runs the output
   projection for its tokens. Batch 0 in one AllToAll (256-token blocks,
   overlaps batch 1); batch 1 in two half-batch AllToAlls (128-token
   blocks) so only the second half sits in the tail.
 - PE p-state warmup: dummy matmuls ramp the tensor engine to full clock
   while the first input DMAs land.
Output per core j: [512, 1024] fp32 - rows 0:256 = batch0 tokens 256j..,
rows 256:384 = batch1 tokens 128j.., rows 384:512 = batch1 tokens
1024+128j..; host reassembles. Matmuls in bf16 (fp32 PSUM accumulation);
softmax in fp32. Post passes: ldweights dedup + splitting multi-wait
instructions into single-wait NoOps (this walrus build caps HW waits at 1).
"""

import numpy as np
import ml_dtypes

import concourse.bass as bass
import concourse.mybir as mybir
import concourse.tile as tile
from concourse.bass_utils import run_bass_kernel_spmd

BF16 = mybir.dt.bfloat16
F32 = mybir.dt.float32
AF = mybir.ActivationFunctionType

B, S, D, H = 2, 2048, 1024, 16
NT = B * S          # 4096 tokens, b-major
NCORES = 8
DK = D // H         # 64
SCALE = 0.125       # 1/sqrt(64)

_CACHE = {}
_NO_COLLECTIVE = False


def _build(debug_dumps=False):
    nc = bass.Bass("TRN2", target_bir_lowering=False, debug=False,
                   num_devices=NCORES)

    xT = nc.dram_tensor("xT", [D, NT], BF16, kind="ExternalInput").ap()
    wqkv = nc.dram_tensor("wqkv", [D, 384], BF16, kind="ExternalInput").ap()
    wp = nc.dram_tensor("wp", [D, D], BF16, kind="ExternalInput").ap()
    cbf16 = nc.dram_tensor("cbf16", [128, 257], BF16, kind="ExternalInput").ap()
    cf32 = nc.dram_tensor("cf32", [128, 3], F32, kind="ExternalInput").ap()
    out = nc.dram_tensor("out", [512, 1024], BF16, kind="ExternalOutput").ap()
    dbg = None
    if debug_dumps:
        dbg = {
            "dbg_qkvT": nc.dram_tensor(
                "dbg_qkvT", [128, 3, NT], BF16, kind="ExternalOutput").ap(),
            "dbg_av": nc.dram_tensor(
                "dbg_av", [128, 32, 128], BF16, kind="ExternalOutput").ap(),
            "dbg_aT": nc.dram_tensor(
                "dbg_aT", [128, 8, 512], BF16, kind="ExternalOutput").ap(),
        }

    with tile.TileContext(nc) as tc:
        _body(tc, out, xT, wqkv, wp, cbf16, cf32, dbg)
    _dedup_ldweights(nc)
    _split_multi_waits(nc)
    return nc


def _dedup_ldweights(nc):
    """Drop a back-to-back identical, wait-free Ldweights (weights already
    resident; only Matmults in between; transposes clobber -> reset)."""
    for f in nc.m.functions:
        for bb in f.blocks:
            insts = bb.instructions
            new = []
            changed = False
            last_w = None
            for inst in insts:
                nm = inst.__class__.__name__
                if getattr(inst, "engine", None) == mybir.EngineType.PE:
                    if nm == "InstLdweights":
                        si = inst.sync_info
                        key = repr(inst.ins)
                        no_waits = si is None or not si.on_wait
                        no_upd = si is None or not si.on_update
                        if key == last_w and no_waits and no_upd:
                            changed = True
                            continue  # drop duplicate load
                        last_w = key
                    elif nm == "InstMatmult":
                        if getattr(inst, "is_transpose", False):
                            last_w = None
                    else:
                        last_w = None
                new.append(inst)
            if changed:
                bb.instructions = new


def _split_multi_waits(nc):
    """Walrus caps HW sync waits at 1 per instruction: hoist extras onto
    dedicated NoOps inserted just before the offender (same engine queue)."""
    import bass_rust
    nid = [0]
    for f in nc.m.functions:
        for bb in f.blocks:
            insts = bb.instructions
            new = []
            changed = False
            for inst in insts:
                si = getattr(inst, "sync_info", None)
                if si is not None and len(si.on_wait) > 1:
                    changed = True
                    waits = list(si.on_wait)
                    for w in waits[:-1]:
                        nid[0] += 1
                        nop = mybir.InstNoOp(
                            name=f"I-waitnop-{nid[0]}", ins=[], outs=[])
                        nop.engine = inst.engine
                        nop.sync_info = bass_rust.SyncInfo(
                            on_wait=[w], on_update=[])
                        new.append(nop)
                    inst.sync_info = bass_rust.SyncInfo(
                        on_wait=[waits[-1]], on_update=list(si.on_update))
                new.append(inst)
            if changed:
                bb.instructions = new


def _body(tc, out, xT, wqkv, wp, cbf16, cf32, dbg=None):
    nc = tc.nc

    with (
        tc.tile_pool(name="persist", bufs=1) as persist,
        tc.tile_pool(name="expp", bufs=24) as expp_pool,
        tc.tile_pool(name="smalls", bufs=4) as smalls_pool,
        tc.tile_pool(name="ablk", bufs=1) as ablk_pool,
        tc.tile_pool(name="outs", bufs=3) as outs_pool,
        tc.tile_pool(name="pss", bufs=2, space="PSUM") as pss_pool,
        tc.tile_pool(name="gemm", bufs=2, space="PSUM") as gemm_pool,
        tc.tile_pool(name="avp", bufs=1, space="PSUM") as avp_pool,
        tc.tile_pool(name="tps", bufs=1, space="PSUM") as tps_pool,
        tc.tile_pool(name="dram", bufs=1, space="DRAM") as dram_pool,
    ):
        # ---- persistent SBUF ----
        xT_sb = persist.tile([128, 8, NT], BF16)        # X^T, D-tile major
        wqkv_sb = persist.tile([128, 8, 384], BF16)
        wp_sb = persist.tile([128, 8, 1024], BF16)
        qkvT_sb = persist.tile([128, 3, NT], BF16)      # q|k|v ^T rows
        v_aug = persist.tile([128, 32, 130], BF16)      # [v_h0|1|v_h1|1] per token-tile
        av_sb = persist.tile([128, 32, 128], BF16)      # normalized av, token-major
        aT_sb = persist.tile([128, 8, 512], BF16)       # a^T after all-to-all
        cbf16_sb = persist.tile([128, 257], BF16)
        cf32_sb = persist.tile([128, 3], F32)
        ident_sb = cbf16_sb[:, 0:128]
        maskmul_sb = cbf16_sb[:, 129:257]   # [k, q]: 1.0 if q >= k else 0.0
        bqkv_sb = cf32_sb[:, 0:3]

        av_bounce = {0: dram_pool.tile([S, 128], BF16, name="avb0"),
                     "1a": dram_pool.tile([S // 2, 128], BF16, name="avb1a"),
                     "1b": dram_pool.tile([S // 4, 128], BF16, name="avb1b"),
                     "1c": dram_pool.tile([S // 4, 128], BF16, name="avb1c")}
        recv_bounce = {0: dram_pool.tile([S, 128], BF16, name="rcv0"),
                       "1a": dram_pool.tile([S // 2, 128], BF16, name="rcv1a"),
                       "1b": dram_pool.tile([S // 4, 128], BF16, name="rcv1b"),
                       "1c": dram_pool.tile([S // 4, 128], BF16, name="rcv1c")}

        # ones columns of v_aug (rest overwritten by V transposes)
        nc.vector.memset(v_aug[:, :, 64:65], 1.0)
        nc.vector.memset(v_aug[:, :, 129:130], 1.0)

        # ---- ACT warmup: attach table-load pseudos to wait-free instructions
        warm = smalls_pool.tile([1, 2], F32, tag="warm")
        nc.vector.memset(warm[:, 0:1], 0.0)
        nc.scalar.activation(warm[:, 1:2], warm[:, 0:1], AF.Identity)
        nc.scalar.activation(warm[:, 1:2], warm[:, 0:1], AF.Exp)
        nc.scalar.activation(warm[:, 1:2], warm[:, 0:1], AF.Copy)

        # ---- slot-rotated persistent PSUM tiles (bank-granular pool slots
        # would otherwise blow the 8-bank budget)
        avps = avp_pool.tile([128, 7, 65], F32)     # AV psum, 7 slots
        tpss = tps_pool.tile([128, 4, 128], BF16)   # transpose psum, 4 slots
        wdum = avps[0:64, 6, 0:64]   # warm-keeping dummy target (reserved)
        av_slot = [0]
        tp_slot = [0]

        # ---- PE p-state warmup: ramp the tensor engine to full clock on
        # dummy matmuls while the first input DMAs land (ramp model: full
        # speed after 3us of continuous execution).
        wtile = smalls_pool.tile([128, 64], BF16, tag="wtile")
        nc.vector.memset(wtile[:, :], 0.0)
        wps = gemm_pool.tile([128, 512], F32, tag="gemm")
        for i in range(48):
            nc.tensor.matmul(wps[0:64, 0:64], wtile[:, 0:64], wtile[:, :],
                             start=True, stop=True)

        # ---- input DMAs. First QKV matmuls need wqkv + xT chunk 0; split
        # those by kt-pairs so early k-tiles land first.
        wqkv_r = wqkv.rearrange("(kt p) n -> p kt n", p=128)
        xT0 = xT[:, 0:512].rearrange("(kt p) w -> p kt w", p=128)
        nc.sync.dma_start(wqkv_sb[:, 0:2, :], wqkv_r[:, 0:2, :])
        nc.sync.dma_start(xT_sb[:, 0:2, 0:512], xT0[:, 0:2, :])
        nc.sync.dma_start(cf32_sb[:, :], cf32[:, :])
        nc.sync.dma_start(cbf16_sb[:, :], cbf16[:, :])
        for kt in range(2, 8, 2):
            nc.sync.dma_start(wqkv_sb[:, kt:kt + 2, :], wqkv_r[:, kt:kt + 2, :])
            nc.sync.dma_start(xT_sb[:, kt:kt + 2, 0:512], xT0[:, kt:kt + 2, :])
        for n in range(1, 8):
            nc.sync.dma_start(
                xT_sb[:, :, n * 512:(n + 1) * 512],
                xT[:, n * 512:(n + 1) * 512]
                .rearrange("(kt p) w -> p kt w", p=128))
        nc.sync.dma_start(wp_sb[:, :, :],
                          wp.rearrange("(kt p) n -> p kt n", p=128))

        def qkv_chunk(n):
            # qkvT[:, :, n*512:(n+1)*512] = Wshard^T @ X^T chunk + bias
            for m in range(3):
                ps = gemm_pool.tile([128, 512], F32, tag="gemm")
                for kt in range(8):
                    nc.tensor.matmul(
                        ps[:, :],
                        wqkv_sb[:, kt, m * 128:(m + 1) * 128],
                        xT_sb[:, kt, n * 512:(n + 1) * 512],
                        start=(kt == 0), stop=(kt == 7),
                    )
                nc.vector.tensor_scalar_add(
                    qkvT_sb[:, m, n * 512:(n + 1) * 512],
                    ps[:, :], bqkv_sb[:, m:m + 1])
            # V transposes -> natural layout, ones-augmented. All four
            # transposes before the copies (bank-granular PSUM deps).
            pts = []
            for t in range(n * 4, n * 4 + 4):
                sl = tp_slot[0] % 4
                tp_slot[0] += 1
                ps_t = tpss[:, sl, :]
                nc.tensor.transpose(
                    ps_t, qkvT_sb[:, 2, t * 128:(t + 1) * 128],
                    ident_sb[:, :])
                pts.append(ps_t)
            for t, ps_t in zip(range(n * 4, n * 4 + 4), pts):
                # both head blocks in one strided copy (ones col at 64 kept)
                nc.vector.tensor_copy(
                    v_aug[:, t:t + 1, 0:130]
                    .rearrange("p a (h q) -> p (a h) q", h=2)[:, :, 0:64],
                    ps_t.rearrange("p (h q) -> p h q", h=2))

        def attn_chunk(b, c):
            tok0 = b * S
            q0 = tok0 + c * 512
            nk = 4 * c + 4
            ets = []

            def av_matmuls(h, qt):
                # flipped AV: stationary = prob tile, moving = [v_h | 1];
                # psum col 64 = softmax denominator per query partition.
                gq = 4 * c + qt
                sl = av_slot[0] % 6   # slot 6 reserved for warm-keeping
                av_slot[0] += 1
                ps_av = avps[:, sl, :]
                for ki in range(gq + 1):
                    et, off = ets[ki]
                    col0 = qt * 128 - off
                    nc.tensor.matmul(
                        ps_av,
                        et[:, h, col0:col0 + 128],
                        v_aug[:, b * 16 + ki, h * 65:(h + 1) * 65],
                        start=(ki == 0), stop=(ki == gq),
                    )
                return ps_av

            def av_normalize(ps_av, h, qt):
                gq = 4 * c + qt
                rec = smalls_pool.tile([128, 1], F32, tag="rec")
                nc.vector.reciprocal(rec[:, :], ps_av[:, 64:65])
                nc.vector.tensor_scalar_mul(
                    av_sb[:, b * 16 + gq, h * 64:(h + 1) * 64],
                    ps_av[:, 0:64], rec[:, :])

            # scores (S^T layout) + exp per k-tile; post-exp causal multiply.
            # AV group (h, qt) is emitted as soon as its last prob tile
            # (diagonal ki = 4c + qt) exists, so only the last groups wait
            # on the final exp of the chunk.
            for ki in range(nk):
                off = max(0, (ki - 4 * c)) * 128
                w = 512 - off
                ps_s = pss_pool.tile([128, 2, 512], F32, tag="pss")
                for h in range(2):
                    hp = h * 64
                    nc.tensor.matmul(
                        ps_s[:, h, :w],
                        qkvT_sb[hp:hp + 64, 1,
                                tok0 + ki * 128: tok0 + (ki + 1) * 128],
                        qkvT_sb[hp:hp + 64, 0, q0 + off: q0 + 512],
                        start=True, stop=True,
                    )
                et = expp_pool.tile([128, 2, 512], BF16, tag="expp")
                nc.scalar.activation(
                    et[:, :, :w], ps_s[:, :, :w], AF.Exp, scale=SCALE)
                if ki >= 4 * c:  # diagonal tile: zero masked (q < k) probs
                    # one head per engine so both mults run in parallel
                    nc.vector.tensor_mul(
                        et[:, 0, 0:128], et[:, 0, 0:128], maskmul_sb)
                    nc.gpsimd.tensor_mul(
                        et[:, 1, 0:128], et[:, 1, 0:128], maskmul_sb)
                ets.append((et, off))
                if ki >= 4 * c:
                    # both heads' AV matmul groups before either normalize:
                    # PSUM dep tracking is bank-granular, so a group's first
                    # (start=True) write would otherwise stall on the
                    # previous group's normalize reads of the shared bank.
                    qt = ki - 4 * c
                    ps0 = av_matmuls(0, qt)
                    ps1 = av_matmuls(1, qt)
                    av_normalize(ps0, 0, qt)
                    av_normalize(ps1, 1, qt)

        def a2a(key, t0, nt):
            # all-to-all over av token-tiles [t0, t0+nt), p-major rows
            # (row = p*nt + t~): block j <=> partitions [16j, 16j+16), i.e.
            # core j receives tokens with t%128 in [16j, 16j+16). p-major
            # keeps both bounce DMAs fully contiguous per partition.
            nc.sync.dma_start(
                av_bounce[key].rearrange("(p t) d -> p t d", t=nt),
                av_sb[:, t0:t0 + nt, :])
            if _NO_COLLECTIVE:
                nc.sync.dma_start(recv_bounce[key][:, :], av_bounce[key][:, :])
            else:
                nc.gpsimd.collective_compute(
                    "AllToAll", mybir.AluOpType.bypass,
                    replica_groups=[list(range(NCORES))],
                    ins=[av_bounce[key][:, :].opt()],
                    outs=[recv_bounce[key][:, :].opt()],
                )

        a_stgs = {}

        def recv_dma(key, nt):
            # issue the recv read as soon as the collective output exists so
            # it never queues behind a later, still-blocked bounce DMA
            a_stg = ablk_pool.tile([128, nt, 128], BF16, tag=f"ablk{key}",
                                   name=f"astg{key}")
            nc.sync.dma_start(
                a_stg[:, :, :],
                recv_bounce[key].rearrange("(k p) d -> p k d", p=128))
            a_stgs[key] = a_stg

        def recv_work(key, nt, col0):
            # rebuild a^T at aT_sb cols [col0, col0 + nt*128/spt). src block
            # = nt*16 rows; a_stg tile k spans 128 rows.
            a_stg = a_stgs[key]
            for k0 in range(0, nt, 4):     # groups of 4 = tpss slot count
                pts = []
                for k in range(k0, min(k0 + 4, nt)):
                    sl = tp_slot[0] % 4
                    tp_slot[0] += 1
                    ps_t = tpss[:, sl, :]
                    nc.tensor.transpose(ps_t, a_stg[:, k, :], ident_sb[:, :])
                    pts.append((k, ps_t))
                for k, ps_t in pts:
                    if nt >= 8:
                        spt = nt // 8      # a_stg tiles per src block
                        s, j = k // spt, k % spt
                        nc.vector.tensor_copy(
                            aT_sb[:, s, col0 + j * 128: col0 + (j + 1) * 128],
                            ps_t)
                    else:                  # two 64-row src blocks per tile
                        nc.vector.tensor_copy(
                            aT_sb[:, 2 * k:2 * k + 2, col0:col0 + 64],
                            ps_t.rearrange("p (s q) -> p s q", s=2))

        def proj(b, mts=(0, 1)):
            for mt in mts:
                r0 = b * 256 + mt * 128
                for n2 in range(2):
                    ps = gemm_pool.tile([128, 512], F32, tag="gemm")
                    for s in range(8):
                        nc.tensor.matmul(
                            ps[:, :],
                            aT_sb[:, s, r0:r0 + 128],
                            wp_sb[:, s, n2 * 512:(n2 + 1) * 512],
                            start=(s == 0), stop=(s == 7),
                        )
                    o_sb = outs_pool.tile([128, 512], BF16, tag="outs")
                    nc.vector.tensor_copy(o_sb[:, :], ps[:, :])
                    nc.sync.dma_start(
                        out[r0:r0 + 128, n2 * 512:(n2 + 1) * 512], o_sb[:, :])

        # ---- main pipeline: QKV chunk n feeds attention chunk (b, c) ----
        for n in range(8):
            qkv_chunk(n)
            b, c = divmod(n, 4)
            attn_chunk(b, c)
        # tile_wait_until stamps pin the scheduler's SP-queue order for the
        # a2a chains (its internal sim otherwise reorders them and the
        # DMA-completion counters then serialize every chain into the tail).
            if (b, c) == (0, 3):
                a2a(0, 0, 16)
                recv_dma(0, 16)
            elif (b, c) == (1, 1):
                with tc.tile_wait_until(0.100):
                    a2a("1a", 16, 8)
                    recv_dma("1a", 8)
            elif (b, c) == (1, 2):
                a2a("1b", 24, 4)       # chunk (1,2): chain hides under (1,3)
                recv_dma("1b", 4)
                recv_work(0, 16, 0)
                proj(0)
        a2a("1c", 28, 4)
        recv_dma("1c", 4)
        recv_work("1a", 8, 256)
        proj(1, mts=(0,))
        # keep the PE clock ramped through the 1c bounce->collective->read
        # chain (~7us): an idle tensor engine drops to the slow p-state and
        # the tail proj would then run 2-4x slow for its first 3us. Dummy
        # transposes target a dedicated psum slot nothing reads.
        recv_work("1b", 4, 384)
        for i in range(240):
            nc.tensor.matmul(wdum, wtile[:, 0:64], wtile[:, :],
                             start=True, stop=True)
        recv_work("1c", 4, 448)
        proj(1, mts=(1,))

        if dbg is not None:
            nc.sync.dma_start(
                dbg["dbg_qkvT"].rearrange("p m n -> p (m n)"),
                qkvT_sb[:, :, :].rearrange("p m n -> p (m n)"))
            nc.sync.dma_start(
                dbg["dbg_av"].rearrange("p t d -> p (t d)"),
                av_sb[:, :, :].rearrange("p t d -> p (t d)"))
            nc.sync.dma_start(
                dbg["dbg_aT"].rearrange("p s n -> p (s n)"),
                aT_sb[:, :, :].rearrange("p s n -> p (s n)"))


def _prep_inputs(hidden_states, c_attn_w, c_attn_b, c_proj_w):
    bf16 = ml_dtypes.bfloat16
    x = np.asarray(hidden_states, dtype=np.float32).reshape(NT, D)
    xT = np.ascontiguousarray(x.T).astype(bf16)
    wp = np.ascontiguousarray(np.asarray(c_proj_w, dtype=np.float32)).astype(bf16)
    identity = np.eye(128, dtype=np.float32)
    ones = np.ones((128, 1), dtype=np.float32)
    # maskmul[k, q] (S^T diagonal tile): keep iff q >= k
    p = np.arange(128)
    maskmul = (p[None, :] >= p[:, None]).astype(np.float32)
    cbf16 = np.ascontiguousarray(
        np.concatenate([identity, ones, maskmul], axis=1)).astype(bf16)

    w = np.asarray(c_attn_w, dtype=np.float32)
    bb = np.asarray(c_attn_b, dtype=np.float32)
    in_maps = []
    for i in range(NCORES):
        cols = np.r_[i * 128:(i + 1) * 128]
        wshard = np.concatenate(
            [w[:, cols], w[:, D + cols], w[:, 2 * D + cols]], axis=1)
        bshard = np.stack(
            [bb[cols], bb[D + cols], bb[2 * D + cols]], axis=1)  # [128, 3]
        cf32 = np.ascontiguousarray(bshard).astype(np.float32)
        in_maps.append({
            "xT": xT,
            "wqkv": np.ascontiguousarray(wshard).astype(bf16),
            "wp": wp,
            "cbf16": cbf16,
            "cf32": cf32,
        })
    return in_maps


def kernel(hidden_states, c_attn_w, c_attn_b, c_proj_w, c_proj_b, _trace=False):
    if "nc" not in _CACHE:
        _CACHE["nc"] = _build()
    nc = _CACHE["nc"]
    in_maps = _prep_inputs(hidden_states, c_attn_w, c_attn_b, c_proj_w)
    try:
        res = run_bass_kernel_spmd(nc, in_maps, core_ids=list(range(NCORES)),
                                   trace=_trace)
    except (ImportError, ModuleNotFoundError):
        # NTFF profiling hook unavailable in this container
        res = run_bass_kernel_spmd(nc, in_maps, core_ids=list(range(NCORES)),
                                   trace=False)
    _CACHE["last_result"] = res
    # p-major a2a blocks: core j owns tokens with t%128 in [16j, 16j+16).
    # row idx within each segment decodes as (pi, t~): token = t~*128+16j+pi.
    idx0 = np.arange(256)
    tok0 = (idx0 % 16) * 128 + (idx0 // 16)        # b0 segment (nt=16)
    idx1 = np.arange(128)
    tok1 = (idx1 % 8) * 128 + (idx1 // 8)          # b1 first half (nt=8)
    idx2 = np.arange(64)
    tok2 = (idx2 % 4) * 128 + (idx2 // 4)          # b1 quarter segs (nt=4)
    full = np.empty((NT, D), dtype=np.float32)
    for j in range(NCORES):
        o = np.asarray(res.results[j]["out"], dtype=np.float32)
        full[tok0 + 16 * j] = o[0:256]
        full[S + tok1 + 16 * j] = o[256:384]
        full[S + 1024 + tok2 + 16 * j] = o[384:448]
        full[S + 1536 + tok2 + 16 * j] = o[448:512]
    full = full + np.asarray(c_proj_b, dtype=np.float32)[None, :]
    return full.reshape(B, S, D).astype(np.float32)


# revision 66
# speedup vs baseline: 1.4288x; 1.0041x over previous
"""GPT-2 attention block on 8 TRN2 NeuronCores.

Sharding (Megatron-style): core i owns heads (2i, 2i+1) for both batches.
 - QKV projection computed transposed: qkvT = Wshard^T @ X^T  -> [384, 4096]
   (rows: q0|q1|k0|k1|v0|v1 head-dim slices, cols: tokens b-major).
   X^T is DMA'd in token chunks; each chunk's projection is immediately
   followed by that chunk's attention work (b, c = divmod(n, 4)) so the
   scalar/vector/pool engines overlap the PE from the start.
 - scores per (batch, head) in transposed layout S^T[k, q], causal tiles
   only, both heads side by side in one 2-bank PSUM tile; one fused exp on
   ScalarE per tile (1/sqrt(64) folded into the activation); causal mask
   applied as a post-exp 0/1 multiply on the diagonal 128x128 block (DVE,
   16-bit 2x mode).
 - AV flipped: stationary = prob tile [128k x 128q], moving = ones-augmented
   V ([v_h | 1], 65 cols) -> psum [128 q, 65] accumulates av AND the softmax
   denominator per query partition; fully-masked (ki > q-tile) matmuls are
   skipped. One fused tensor_scalar divide normalizes av (per-partition
   denominator scalar straight from PSUM col 64) - no transposes, no
   denominator DRAM bounce.
 - AllToAlls reshard to sequence parallelism; each core runs the output
   projection for its tokens. Batch 0 in one AllToAll (256-token blocks,
   overlaps batch 1); batch 1 in two half-batch AllToAlls (128-token
   blocks) so only the second half sits in the tail.
 - PE p-state warmup: dummy matmuls ramp the tensor engine to full clock
   while the first input DMAs land.
Output per core j: [512, 1024] fp32 - rows 0:256 = batch0 tokens 256j..,
rows 256:384 = batch1 tokens 128j.., rows 384:512 = batch1 tokens
1024+128j..; host reassembles. Matmuls in bf16 (fp32 PSUM accumulation);
softmax in fp32. Post passes: ldweights dedup + splitting multi-wait
instructions into single-wait NoOps (this walrus build caps HW waits at 1).
"""

import numpy as np
import ml_dtypes

import concourse.bass as bass
import concourse.mybir as mybir
import concourse.tile as tile
from concourse.bass_utils import run_bass_kernel_spmd

BF16 = mybir.dt.bfloat16
F32 = mybir.dt.float32
AF = mybir.ActivationFunctionType

B, S, D, H = 2, 2048, 1024, 16
NT = B * S          # 4096 tokens, b-major
NCORES = 8
DK = D // H         # 64
SCALE = 0.125       # 1/sqrt(64)

_CACHE = {}
_NO_COLLECTIVE = False


def _build(debug_dumps=False):
    nc = bass.Bass("TRN2", target_bir_lowering=False, debug=False,
                   num_devices=NCORES)

    xT = nc.dram_tensor("xT", [D, NT], BF16, kind="ExternalInput").ap()
    wqkv = nc.dram_tensor("wqkv", [D, 384], BF16, kind="ExternalInput").ap()
    wp = nc.dram_tensor("wp", [D, D], BF16, kind="ExternalInput").ap()
    cbf16 = nc.dram_tensor("cbf16", [128, 257], BF16, kind="ExternalInput").ap()
    cf32 = nc.dram_tensor("cf32", [128, 3], F32, kind="ExternalInput").ap()
    out = nc.dram_tensor("out", [512, 1024], BF16, kind="ExternalOutput").ap()
    dbg = None
    if debug_dumps:
        dbg = {
            "dbg_qkvT": nc.dram_tensor(
                "dbg_qkvT", [128, 3, NT], BF16, kind="ExternalOutput").ap(),
            "dbg_av": nc.dram_tensor(
                "dbg_av", [128, 32, 128], BF16, kind="ExternalOutput").ap(),
            "dbg_aT": nc.dram_tensor(
                "dbg_aT", [128, 8, 512], BF16, kind="ExternalOutput").ap(),
        }

    with tile.TileContext(nc) as tc:
        _body(tc, out, xT, wqkv, wp, cbf16, cf32, dbg)
    _dedup_ldweights(nc)
    _split_multi_waits(nc)
    return nc


def _dedup_ldweights(nc):
    """Drop a back-to-back identical, wait-free Ldweights (weights already
    resident; only Matmults in between; transposes clobber -> reset)."""
    for f in nc.m.functions:
        for bb in f.blocks:
            insts = bb.instructions
            new = []
            changed = False
            last_w = None
            for inst in insts:
                nm = inst.__class__.__name__
                if getattr(inst, "engine", None) == mybir.EngineType.PE:
                    if nm == "InstLdweights":
                        si = inst.sync_info
                        key = repr(inst.ins)
                        no_waits = si is None or not si.on_wait
                        no_upd = si is None or not si.on_update
                        if key == last_w and no_waits and no_upd:
                            changed = True
                            continue  # drop duplicate load
                        last_w = key
                    elif nm == "InstMatmult":
                        if getattr(inst, "is_transpose", False):
                            last_w = None
                    else:
                        last_w = None
                new.append(inst)
            if changed:
                bb.instructions = new


def _split_multi_waits(nc):
    """Walrus caps HW sync waits at 1 per instruction: hoist extras onto
    dedicated NoOps inserted just before the offender (same engine queue)."""
    import bass_rust
    nid = [0]
    for f in nc.m.functions:
        for bb in f.blocks:
            insts = bb.instructions
            new = []
            changed = False
            for inst in insts:
                si = getattr(inst, "sync_info", None)
                if si is not None and len(si.on_wait) > 1:
                    changed = True
                    waits = list(si.on_wait)
                    for w in waits[:-1]:
                        nid[0] += 1
                        nop = mybir.InstNoOp(
                            name=f"I-waitnop-{nid[0]}", ins=[], outs=[])
                        nop.engine = inst.engine
                        nop.sync_info = bass_rust.SyncInfo(
                            on_wait=[w], on_update=[])
                        new.append(nop)
                    inst.sync_info = bass_rust.SyncInfo(
                        on_wait=[waits[-1]], on_update=list(si.on_update))
                new.append(inst)
            if changed:
                bb.instructions = new


def _body(tc, out, xT, wqkv, wp, cbf16, cf32, dbg=None):
    nc = tc.nc

    with (
        tc.tile_pool(name="persist", bufs=1) as persist,
        tc.tile_pool(name="expp", bufs=24) as expp_pool,
        tc.tile_pool(name="smalls", bufs=4) as smalls_pool,
        tc.tile_pool(name="ablk", bufs=1) as ablk_pool,
        tc.tile_pool(name="outs", bufs=3) as outs_pool,
        tc.tile_pool(name="pss", bufs=2, space="PSUM") as pss_pool,
        tc.tile_pool(name="gemm", bufs=2, space="PSUM") as gemm_pool,
        tc.tile_pool(name="avp", bufs=1, space="PSUM") as avp_pool,
        tc.tile_pool(name="tps", bufs=1, space="PSUM") as tps_pool,
        tc.tile_pool(name="dram", bufs=1, space="DRAM") as dram_pool,
    ):
        # ---- persistent SBUF ----
        xT_sb = persist.tile([128, 8, NT], BF16)        # X^T, D-tile major
        wqkv_sb = persist.tile([128, 8, 384], BF16)
        wp_sb = persist.tile([128, 8, 1024], BF16)
        qkvT_sb = persist.tile([128, 3, NT], BF16)      # q|k|v ^T rows
        v_aug = persist.tile([128, 32, 130], BF16)      # [v_h0|1|v_h1|1] per token-tile
        av_sb = persist.tile([128, 32, 128], BF16)      # normalized av, token-major
        aT_sb = persist.tile([128, 8, 512], BF16)       # a^T after all-to-all
        cbf16_sb = persist.tile([128, 257], BF16)
        cf32_sb = persist.tile([128, 3], F32)
        ident_sb = cbf16_sb[:, 0:128]
        maskmul_sb = cbf16_sb[:, 129:257]   # [k, q]: 1.0 if q >= k else 0.0
        bqkv_sb = cf32_sb[:, 0:3]

        av_bounce = {0: dram_pool.tile([S, 128], BF16, name="avb0"),
                     "1a": dram_pool.tile([S // 2, 128], BF16, name="avb1a"),
                     "1b": dram_pool.tile([S // 4, 128], BF16, name="avb1b"),
                     "1c": dram_pool.tile([S // 4, 128], BF16, name="avb1c")}
        recv_bounce = {0: dram_pool.tile([S, 128], BF16, name="rcv0"),
                       "1a": dram_pool.tile([S // 2, 128], BF16, name="rcv1a"),
                       "1b": dram_pool.tile([S // 4, 128], BF16, name="rcv1b"),
                       "1c": dram_pool.tile([S // 4, 128], BF16, name="rcv1c")}

        # warm tile first: the PE p-state warmup matmuls depend only on it
        wtile = smalls_pool.tile([128, 64], BF16, tag="wtile")
        nc.vector.memset(wtile[:, :], 0.0)

        # ones columns of v_aug (rest overwritten by V transposes)
        nc.vector.memset(v_aug[:, :, 64:65], 1.0)
        nc.vector.memset(v_aug[:, :, 129:130], 1.0)

        # ---- ACT warmup: attach table-load pseudos to wait-free instructions
        warm = smalls_pool.tile([1, 2], F32, tag="warm")
        nc.vector.memset(warm[:, 0:1], 0.0)
        nc.scalar.activation(warm[:, 1:2], warm[:, 0:1], AF.Identity)
        nc.scalar.activation(warm[:, 1:2], warm[:, 0:1], AF.Exp)
        nc.scalar.activation(warm[:, 1:2], warm[:, 0:1], AF.Copy)

        # ---- slot-rotated persistent PSUM tiles (bank-granular pool slots
        # would otherwise blow the 8-bank budget)
        avps = avp_pool.tile([128, 7, 65], F32)     # AV psum, 7 slots
        tpss = tps_pool.tile([128, 4, 128], BF16)   # transpose psum, 4 slots
        wdum = avps[0:64, 6, 0:64]   # warm-keeping dummy target (reserved)
        av_slot = [0]
        tp_slot = [0]

        # ---- PE p-state warmup: ramp the tensor engine to full clock on
        # dummy matmuls while the first input DMAs land (ramp model: full
        # speed after 3us of continuous execution).
        wps = gemm_pool.tile([128, 512], F32, tag="gemm")
        for i in range(72):
            nc.tensor.matmul(wps[0:64, 0:64], wtile[:, 0:64], wtile[:, :],
                             start=True, stop=True)

        # ---- input DMAs. First QKV matmuls need wqkv + xT chunk 0; split
        # those by kt-pairs so early k-tiles land first.
        wqkv_r = wqkv.rearrange("(kt p) n -> p kt n", p=128)
        xT0 = xT[:, 0:512].rearrange("(kt p) w -> p kt w", p=128)
        nc.sync.dma_start(wqkv_sb[:, 0:2, :], wqkv_r[:, 0:2, :])
        nc.sync.dma_start(xT_sb[:, 0:2, 0:512], xT0[:, 0:2, :])
        nc.sync.dma_start(cf32_sb[:, :], cf32[:, :])
        nc.sync.dma_start(cbf16_sb[:, :], cbf16[:, :])
        for kt in range(2, 8, 2):
            nc.sync.dma_start(wqkv_sb[:, kt:kt + 2, :], wqkv_r[:, kt:kt + 2, :])
            nc.sync.dma_start(xT_sb[:, kt:kt + 2, 0:512], xT0[:, kt:kt + 2, :])
        for n in range(1, 8):
            nc.sync.dma_start(
                xT_sb[:, :, n * 512:(n + 1) * 512],
                xT[:, n * 512:(n + 1) * 512]
                .rearrange("(kt p) w -> p kt w", p=128))
        nc.sync.dma_start(wp_sb[:, :, :],
                          wp.rearrange("(kt p) n -> p kt n", p=128))

        def qkv_chunk(n):
            # qkvT[:, :, n*512:(n+1)*512] = Wshard^T @ X^T chunk + bias
            for m in range(3):
                ps = gemm_pool.tile([128, 512], F32, tag="gemm")
                for kt in range(8):
                    nc.tensor.matmul(
                        ps[:, :],
                        wqkv_sb[:, kt, m * 128:(m + 1) * 128],
                        xT_sb[:, kt, n * 512:(n + 1) * 512],
                        start=(kt == 0), stop=(kt == 7),
                    )
                nc.vector.tensor_scalar_add(
                    qkvT_sb[:, m, n * 512:(n + 1) * 512],
                    ps[:, :], bqkv_sb[:, m:m + 1])
            # V transposes -> natural layout, ones-augmented. All four
            # transposes before the copies (bank-granular PSUM deps).
            pts = []
            for t in range(n * 4, n * 4 + 4):
                sl = tp_slot[0] % 4
                tp_slot[0] += 1
                ps_t = tpss[:, sl, :]
                nc.tensor.transpose(
                    ps_t, qkvT_sb[:, 2, t * 128:(t + 1) * 128],
                    ident_sb[:, :])
                pts.append(ps_t)
            for t, ps_t in zip(range(n * 4, n * 4 + 4), pts):
                # both head blocks in one strided copy (ones col at 64 kept)
                nc.vector.tensor_copy(
                    v_aug[:, t:t + 1, 0:130]
                    .rearrange("p a (h q) -> p (a h) q", h=2)[:, :, 0:64],
                    ps_t.rearrange("p (h q) -> p h q", h=2))

        def attn_chunk(b, c):
            tok0 = b * S
            q0 = tok0 + c * 512
            nk = 4 * c + 4
            ets = []
            if b == 1:
                # bridge the chunk-boundary exp-drain stall (keeps the PE
                # p-state ramped; these run only when PE would idle)
                for i in range(10):
                    nc.tensor.matmul(wdum, wtile[:, 0:64], wtile[:, :],
                                     start=True, stop=True)

            def av_matmuls(h, qt):
                # flipped AV: stationary = prob tile, moving = [v_h | 1];
                # psum col 64 = softmax denominator per query partition.
                gq = 4 * c + qt
                sl = av_slot[0] % 6   # slot 6 reserved for warm-keeping
                av_slot[0] += 1
                ps_av = avps[:, sl, :]
                for ki in range(gq + 1):
                    et, off = ets[ki]
                    col0 = qt * 128 - off
                    nc.tensor.matmul(
                        ps_av,
                        et[:, h, col0:col0 + 128],
                        v_aug[:, b * 16 + ki, h * 65:(h + 1) * 65],
                        start=(ki == 0), stop=(ki == gq),
                    )
                return ps_av

            def av_normalize(ps_av, h, qt):
                gq = 4 * c + qt
                rec = smalls_pool.tile([128, 1], F32, tag="rec")
                nc.vector.reciprocal(rec[:, :], ps_av[:, 64:65])
                nc.vector.tensor_scalar_mul(
                    av_sb[:, b * 16 + gq, h * 64:(h + 1) * 64],
                    ps_av[:, 0:64], rec[:, :])

            # scores (S^T layout) + exp per k-tile; post-exp causal multiply.
            # AV group (h, qt) is emitted as soon as its last prob tile
            # (diagonal ki = 4c + qt) exists, so only the last groups wait
            # on the final exp of the chunk.
            for ki in range(nk):
                off = max(0, (ki - 4 * c)) * 128
                w = 512 - off
                ps_s = pss_pool.tile([128, 2, 512], F32, tag="pss")
                for h in range(2):
                    hp = h * 64
                    nc.tensor.matmul(
                        ps_s[:, h, :w],
                        qkvT_sb[hp:hp + 64, 1,
                                tok0 + ki * 128: tok0 + (ki + 1) * 128],
                        qkvT_sb[hp:hp + 64, 0, q0 + off: q0 + 512],
                        start=True, stop=True,
                    )
                et = expp_pool.tile([128, 2, 512], BF16, tag="expp")
                nc.scalar.activation(
                    et[:, :, :w], ps_s[:, :, :w], AF.Exp, scale=SCALE)
                if ki >= 4 * c:  # diagonal tile: zero masked (q < k) probs
                    # one head per engine so both mults run in parallel
                    nc.vector.tensor_mul(
                        et[:, 0, 0:128], et[:, 0, 0:128], maskmul_sb)
                    nc.gpsimd.tensor_mul(
                        et[:, 1, 0:128], et[:, 1, 0:128], maskmul_sb)
                ets.append((et, off))
                if ki >= 4 * c:
                    # both heads' AV matmul groups before either normalize:
                    # PSUM dep tracking is bank-granular, so a group's first
                    # (start=True) write would otherwise stall on the
                    # previous group's normalize reads of the shared bank.
                    qt = ki - 4 * c
                    ps0 = av_matmuls(0, qt)
                    ps1 = av_matmuls(1, qt)
                    av_normalize(ps0, 0, qt)
                    av_normalize(ps1, 1, qt)

        def a2a(key, t0, nt):
            # all-to-all over av token-tiles [t0, t0+nt), p-major rows
            # (row = p*nt + t~): block j <=> partitions [16j, 16j+16), i.e.
            # core j receives tokens with t%128 in [16j, 16j+16). p-major
            # keeps both bounce DMAs fully contiguous per partition.
            nc.sync.dma_start(
                av_bounce[key].rearrange("(p t) d -> p t d", t=nt),
                av_sb[:, t0:t0 + nt, :])
            if _NO_COLLECTIVE:
                nc.sync.dma_start(recv_bounce[key][:, :], av_bounce[key][:, :])
            else:
                nc.gpsimd.collective_compute(
                    "AllToAll", mybir.AluOpType.bypass,
                    replica_groups=[list(range(NCORES))],
                    ins=[av_bounce[key][:, :].opt()],
                    outs=[recv_bounce[key][:, :].opt()],
                )

        a_stgs = {}

        def recv_dma(key, nt):
            # issue the recv read as soon as the collective output exists so
            # it never queues behind a later, still-blocked bounce DMA
            a_stg = ablk_pool.tile([128, nt, 128], BF16, tag=f"ablk{key}",
                                   name=f"astg{key}")
            nc.sync.dma_start(
                a_stg[:, :, :],
                recv_bounce[key].rearrange("(k p) d -> p k d", p=128))
            a_stgs[key] = a_stg

        def recv_work(key, nt, col0):
            # rebuild a^T at aT_sb cols [col0, col0 + nt*128/spt). src block
            # = nt*16 rows; a_stg tile k spans 128 rows.
            a_stg = a_stgs[key]
            for k0 in range(0, nt, 4):     # groups of 4 = tpss slot count
                pts = []
                for k in range(k0, min(k0 + 4, nt)):
                    sl = tp_slot[0] % 4
                    tp_slot[0] += 1
                    ps_t = tpss[:, sl, :]
                    nc.tensor.transpose(ps_t, a_stg[:, k, :], ident_sb[:, :])
                    pts.append((k, ps_t))
                for k, ps_t in pts:
                    if nt >= 8:
                        spt = nt // 8      # a_stg tiles per src block
                        s, j = k // spt, k % spt
                        nc.vector.tensor_copy(
                            aT_sb[:, s, col0 + j * 128: col0 + (j + 1) * 128],
                            ps_t)
                    else:                  # two 64-row src blocks per tile
                        nc.vector.tensor_copy(
                            aT_sb[:, 2 * k:2 * k + 2, col0:col0 + 64],
                            ps_t.rearrange("p (s q) -> p s q", s=2))

        def proj(b, mts=(0, 1), tail=False):
            for mt in mts:
                r0 = b * 256 + mt * 128
                # tail mode quarters the last output so the final
                # copy->DMA chain carries 256 cols instead of 512
                nw = 256 if tail and mt == mts[-1] else 512
                for n0 in range(0, 1024, nw):
                    ps = gemm_pool.tile([128, 512], F32, tag="gemm")
                    for s in range(8):
                        nc.tensor.matmul(
                            ps[:, 0:nw],
                            aT_sb[:, s, r0:r0 + 128],
                            wp_sb[:, s, n0:n0 + nw],
                            start=(s == 0), stop=(s == 7),
                        )
                    o_sb = outs_pool.tile([128, 512], BF16, tag="outs")
                    nc.vector.tensor_copy(o_sb[:, 0:nw], ps[:, 0:nw])
                    nc.sync.dma_start(
                        out[r0:r0 + 128, n0:n0 + nw], o_sb[:, 0:nw])

        # ---- main pipeline: QKV chunk n feeds attention chunk (b, c) ----
        for n in range(8):
            qkv_chunk(n)
            b, c = divmod(n, 4)
            attn_chunk(b, c)
        # tile_wait_until stamps pin the scheduler's SP-queue order for the
        # a2a chains (its internal sim otherwise reorders them and the
        # DMA-completion counters then serialize every chain into the tail).
            if (b, c) == (0, 3):
                a2a(0, 0, 16)
                recv_dma(0, 16)
            elif (b, c) == (1, 1):
                with tc.tile_wait_until(0.100):
                    a2a("1a", 16, 8)
                    recv_dma("1a", 8)
            elif (b, c) == (1, 2):
                a2a("1b", 24, 4)       # chunk (1,2): chain hides under (1,3)
                recv_dma("1b", 4)
                recv_work(0, 16, 0)
                proj(0)
        a2a("1c", 28, 4)
        recv_dma("1c", 4)
        recv_work("1a", 8, 256)
        proj(1, mts=(0,))
        # keep the PE clock ramped through the 1c bounce->collective->read
        # chain (~7us): an idle tensor engine drops to the slow p-state and
        # the tail proj would then run 2-4x slow for its first 3us. Dummy
        # transposes target a dedicated psum slot nothing reads.
        recv_work("1b", 4, 384)
        for i in range(240):
            nc.tensor.matmul(wdum, wtile[:, 0:64], wtile[:, :],
                             start=True, stop=True)
        recv_work("1c", 4, 448)
        proj(1, mts=(1,))

        if dbg is not None:
            nc.sync.dma_start(
                dbg["dbg_qkvT"].rearrange("p m n -> p (m n)"),
                qkvT_sb[:, :, :].rearrange("p m n -> p (m n)"))
            nc.sync.dma_start(
                dbg["dbg_av"].rearrange("p t d -> p (t d)"),
                av_sb[:, :, :].rearrange("p t d -> p (t d)"))
            nc.sync.dma_start(
                dbg["dbg_aT"].rearrange("p s n -> p (s n)"),
                aT_sb[:, :, :].rearrange("p s n -> p (s n)"))


def _prep_inputs(hidden_states, c_attn_w, c_attn_b, c_proj_w):
    bf16 = ml_dtypes.bfloat16
    x = np.asarray(hidden_states, dtype=np.float32).reshape(NT, D)
    xT = np.ascontiguousarray(x.T).astype(bf16)
    wp = np.ascontiguousarray(np.asarray(c_proj_w, dtype=np.float32)).astype(bf16)
    identity = np.eye(128, dtype=np.float32)
    ones = np.ones((128, 1), dtype=np.float32)
    # maskmul[k, q] (S^T diagonal tile): keep iff q >= k
    p = np.arange(128)
    maskmul = (p[None, :] >= p[:, None]).astype(np.float32)
    cbf16 = np.ascontiguousarray(
        np.concatenate([identity, ones, maskmul], axis=1)).astype(bf16)

    w = np.asarray(c_attn_w, dtype=np.float32)
    bb = np.asarray(c_attn_b, dtype=np.float32)
    in_maps = []
    for i in range(NCORES):
        cols = np.r_[i * 128:(i + 1) * 128]
        wshard = np.concatenate(
            [w[:, cols], w[:, D + cols], w[:, 2 * D + cols]], axis=1)
        bshard = np.stack(
            [bb[cols], bb[D + cols], bb[2 * D + cols]], axis=1)  # [128, 3]
        cf32 = np.ascontiguousarray(bshard).astype(np.float32)
        in_maps.append({
            "xT": xT,
            "wqkv": np.ascontiguousarray(wshard).astype(bf16),
            "wp": wp,
            "cbf16": cbf16,
            "cf32": cf32,
        })
    return in_maps


def kernel(hidden_states, c_attn_w, c_attn_b, c_proj_w, c_proj_b, _trace=False):
    if "nc" not in _CACHE:
        _CACHE["nc"] = _build()
    nc = _CACHE["nc"]
    in_maps = _prep_inputs(hidden_states, c_attn_w, c_attn_b, c_proj_w)
    try:
        res = run_bass_kernel_spmd(nc, in_maps, core_ids=list(range(NCORES)),
                                   trace=_trace)
    except (ImportError, ModuleNotFoundError):
        # NTFF profiling hook unavailable in this container
        res = run_bass_kernel_spmd(nc, in_maps, core_ids=list(range(NCORES)),
                                   trace=False)
    _CACHE["last_result"] = res
    # p-major a2a blocks: core j owns tokens with t%128 in [16j, 16j+16).
    # row idx within each segment decodes as (pi, t~): token = t~*128+16j+pi.
    idx0 = np.arange(256)
    tok0 = (idx0 % 16) * 128 + (idx0 // 16)        # b0 segment (nt=16)
    idx1 = np.arange(128)
    tok1 = (idx1 % 8) * 128 + (idx1 // 8)          # b1 first half (nt=8)
    idx2 = np.arange(64)
    tok2 = (idx2 % 4) * 128 + (idx2 // 4)          # b1 quarter segs (nt=4)
    full = np.empty((NT, D), dtype=np.float32)
    for j in range(NCORES):
        o = np.asarray(res.results[j]["out"], dtype=np.float32)
        full[tok0 + 16 * j] = o[0:256]
        full[S + tok1 + 16 * j] = o[256:384]
        full[S + 1024 + tok2 + 16 * j] = o[384:448]
        full[S + 1536 + tok2 + 16 * j] = o[448:512]
    full = full + np.asarray(c_proj_b, dtype=np.float32)[None, :]
    return full.reshape(B, S, D).astype(np.float32)
